# revision 24
# baseline (speedup 1.0000x reference)
"""Trainium2 Bass kernel for nn_AdaptiveBoundaryRefinement_45861660787095.

Self-contained: takes FULL inputs (B=16,M=128,T=12000), shards batch across 8
NeuronCores (2 samples/core), runs a Bass/Tile kernel per core, gathers.

v3 design (v2 tail numerics kept verbatim; front-end rebuilt from trace data):
- mel streams as RAW F32 over two DMA queues (sync HWDGE + gpsimd SWDGE,
  alternating tiles).  The v2 f32->bf16 cast-DMA saturated all 16 SDMA
  engines at only ~280 GB/s read-side; plain f32 streams at the ~358 GB/s
  HBM ceiling and frees the cast entirely.
- S (column sums) matmuls stream the f32 data directly as f32r: with moving
  dim >=256 the PE streams f32r at 1 cycle/row, so no bf16 copy of mel is
  needed anywhere.
- sq = mel^2 moves to the Scalar engine (ACT Square, f32-in/bf16-out,
  1 elem/lane/cycle) - the DVE was the measured critical resource (50.5us
  busy, 54% occ) and TT tops out at 2x regardless of dtype.
- cross = mel[t]*mel[t+1] stays on DVE (f32 1x) except a few big tiles that
  run on GpSimd, balancing all three elementwise engines under the stream.
- Q and D fuse into ONE matmul per chunk: sq and cross live in one combined
  SBUF buffer (cross at column offset CA) and the rhs is a 2-block strided
  AP [[CA,2],[1,261]]; the out AP [[512,2],[1,261]] writes the Q bank and D
  bank of a 2-bank PSUM tile in one pass.  Halves the MM count -> fewer
  fixed overheads and a denser PE stream (keeps the p-state high).
- Per-sample tail (win5 sums, cos, thresholds, closed-form 5-iteration
  refinement via predicated copies) is byte-identical to the validated v2.
- The batch-global early-stop of the reference is a mathematical no-op.
"""

import os
import sys

import numpy as np

_TRN_REPO = "/opt/trn_rl_repo"
if _TRN_REPO not in sys.path:
    sys.path.insert(0, _TRN_REPO)

import concourse.bass as bass
import concourse.bacc as bacc
import concourse.mybir as mybir
import concourse.tile as tile
from concourse.bass_utils import run_bass_kernel_spmd

F32 = mybir.dt.float32
F32R = mybir.dt.float32r
BF16 = mybir.dt.bfloat16
ALU = mybir.AluOpType
ACTF = mybir.ActivationFunctionType
AX = mybir.AxisListType

B, M, T = 16, 128, 12000
NCORES = 8
BPC = B // NCORES            # samples per core = 2
CH = 256                     # chunk width
NCH = (T + CH - 1) // CH     # 47 chunks per sample
EPS2 = 1e-12
GRAD_THRESH = 0.15
LASTW = T - CH * (NCH - 1)   # 224 real cols in the last chunk
NGS = [1, 4, 8, 8, 8, 8, 8, 2]  # chunks per mel tile (small ramp/tail tiles)
SBASE = (0, 64)              # partition base of each sample's chunk rows
NROWS = SBASE[1] + NCH       # 111
CA = 2056                    # cross region offset inside the combined qd tile
QDW = CA + 2053 + 3          # combined sq|cross tile width (bf16)
# odd-j tiles ride the gpsimd (SWDGE) queue and are cast f32->bf16 during the
# DMA: their cross runs at DVE 2x and their S-matmul streams bf16.  Even-j
# tiles stay f32 on the sync (HWDGE) queue at full HBM rate.


def _softmax_f32(x):
    x = np.asarray(x, np.float32)
    m = np.max(x).astype(np.float32)
    e = np.exp((x - m).astype(np.float32)).astype(np.float32)
    return (e / e.sum(dtype=np.float32).astype(np.float32)).astype(np.float32)


def _const_masks(w0, w1):
    import ml_dtypes

    bf = ml_dtypes.bfloat16
    # w1vC [NROWS, 260]: w1 where col maps to t in [0, T), else 0
    # (col h of chunk p -> t = 256p - 2 + h).  Also t=0 keeps w1 (spec_sim=1
    # there via the specH shift trick).
    w1vC = np.full((NROWS, 260), np.float32(w1), np.float32)
    for r0 in SBASE:
        w1vC[r0, 0:2] = 0.0                       # t < 0
        w1vC[r0 + NCH - 1, LASTW + 2 : 260] = 0.0  # t >= T
    # SMb [NROWS, 256]: 0.2 interior, 0.25 at t=1 and t=T-2 (win4 edges)
    SM = np.full((NROWS, 256), np.float32(0.2), np.float32)
    for r0 in SBASE:
        SM[r0, 1] = 0.25
        SM[r0 + NCH - 1, LASTW - 2] = 0.25
    # gate01 [NROWS, 256]: 1 for t in [1, T-2], else 0
    gate = np.ones((NROWS, 256), np.float32)
    for r0 in SBASE:
        gate[r0, 0] = 0.0
        gate[r0 + NCH - 1, LASTW - 1 : 256] = 0.0
    # wzb one-hot bank [128, 257]: col 128 is ones; slice [128-j : 256-j]
    # puts the hot column at position j (j in 0..127).
    wzb = np.zeros((128, 257), bf)
    wzb[:, 128] = 1.0
    wz32 = np.zeros((128, 257), np.float32)
    wz32[:, 128] = 1.0
    # indB [128, 2*NROWS]: cols [0:NROWS] sample-0 block-indicator lhsT,
    # cols [NROWS:2*NROWS] sample-1.  out[i,:] = sum_{p in sample} W[p,:]
    # lands on every row i of that sample -> per-partition broadcast sums.
    indB = np.zeros((128, 2 * NROWS), bf)
    indB[0:NCH, 0:NCH] = 1.0
    indB[SBASE[1] : SBASE[1] + NCH, NROWS + SBASE[1] : NROWS + SBASE[1] + NCH] = 1.0
    return w1vC.astype(bf), SM.astype(bf), gate.astype(bf), wzb, indB, wz32


def _host_pack(spec2, init2, w1):
    """Row-pack the [BPC,T] side inputs into the kernel's [NROWS,*] layouts.

    All arithmetic in f32 to mirror the device ops this replaces."""
    rpk = np.zeros((NROWS, 256), np.float32)
    t1 = np.zeros((NROWS, 260), np.float32)
    g05 = np.zeros((NROWS, 256), np.float32)
    rU = np.zeros((NROWS, 256), np.float32)
    rDA = np.zeros((NROWS, 256), np.float32)
    rD0 = np.zeros((NROWS, 256), np.float32)
    for b in range(BPC):
        r0 = SBASE[b]
        full = np.zeros(NCH * 256, np.float32)
        full[:T] = init2[b]
        R = full.reshape(NCH, 256)
        rpk[r0 : r0 + NCH] = R
        g05[r0 : r0 + NCH] = (R > np.float32(0.5)).astype(np.float32)
        rU[r0 : r0 + NCH] = np.minimum(R + np.float32(0.5), np.float32(1.0))
        rDA[r0 : r0 + NCH] = np.maximum(R - np.float32(0.5), np.float32(0.0))
        y = R * np.float32(10.0) - np.float32(5.0)
        k = np.zeros_like(R)
        for j in range(5):
            k += (y > np.float32(j)).astype(np.float32)
        rD0[r0 : r0 + NCH] = R + np.float32(-0.1) * k
        # t1sv: col h of chunk row rl -> t = 256*rl - 2 + h; w1/(1+|dspec|),
        # spec_sim(0)=1, zero outside [0, T)
        sp = spec2[b]
        ssim = np.ones(T, np.float32)
        ssim[1:] = np.float32(1.0) / (
            np.float32(1.0) + np.abs(sp[1:] - sp[:-1]).astype(np.float32)
        )
        tv = np.zeros(NCH * 256 + 260, np.float32)
        tv[2 : 2 + T] = np.float32(w1) * ssim
        for rl in range(NCH):
            t1[r0 + rl] = tv[256 * rl : 256 * rl + 260]
    return rpk, t1, g05, rU, rDA, rD0


def build_nc(w0, w1, w2):
    nc = bacc.Bacc("TRN2", target_bir_lowering=False, debug=False)
    mel = nc.dram_tensor("mel_features", [BPC, M, T], F32, kind="ExternalInput")
    # host-precomputed, row-packed [NROWS, *] side inputs (see _host_pack)
    rpk_d = nc.dram_tensor("rpk", [NROWS, 256], F32, kind="ExternalInput")
    t1sv_d = nc.dram_tensor("t1svh", [NROWS, 260], BF16, kind="ExternalInput")
    g05_d = nc.dram_tensor("g05h", [NROWS, 256], BF16, kind="ExternalInput")
    rU_d = nc.dram_tensor("rUh", [NROWS, 256], F32, kind="ExternalInput")
    rDA_d = nc.dram_tensor("rDAh", [NROWS, 256], F32, kind="ExternalInput")
    rD0_d = nc.dram_tensor("rD0h", [NROWS, 256], F32, kind="ExternalInput")
    out = nc.dram_tensor("out", [BPC, T], F32, kind="ExternalOutput")
    KDBG = bool(os.environ.get("KDBG"))
    dbg = {}
    if KDBG:
        for nm, w, dt in (
            ("dbg_W", 256, BF16), ("dbg_thr", 2, F32), ("dbg_local", 256, BF16),
            ("dbg_consH", 260, BF16), ("dbg_A", 256, BF16), ("dbg_Sb", 260, BF16),
            ("dbg_Qs", 261, BF16), ("dbg_cos", 260, BF16), ("dbg_t1sv", 260, BF16),
            ("dbg_Ds", 260, F32), ("dbg_den2", 260, BF16), ("dbg_sd", 260, F32),
            ("dbg_rs", 260, F32),
        ):
            dbg[nm] = nc.dram_tensor(nm, [BPC, NROWS, w], dt, kind="ExternalOutput")

    _w1vC_np, SM_np, gate_np, wzb_np, indB_np, wz32_np = _const_masks(w0, w1)
    wzb_d = nc.inline_tensor(wzb_np, name="wzbc")
    wz32_d = nc.inline_tensor(wz32_np, name="wz32c")
    indB_d = nc.inline_tensor(indB_np, name="indB")
    SM_d = nc.inline_tensor(SM_np, name="SMb")
    gate_d = nc.inline_tensor(gate_np, name="gate01")

    th2 = float(np.float32(GRAD_THRESH) * np.float32(GRAD_THRESH))
    SMSC = 0.2 / 128.0

    with tile.TileContext(nc) as tc:
        with (
            tc.tile_pool(name="melf", bufs=6) as pmelf,
            tc.tile_pool(name="melc", bufs=6) as pmelc,
            tc.tile_pool(name="qd", bufs=6) as pqd,
            tc.tile_pool(name="stat", bufs=1) as pstat,
            tc.tile_pool(name="ps", bufs=1, space="PSUM") as pps,
            tc.tile_pool(name="ps2", bufs=1, space="PSUM") as pps2,
        ):
            # ---------------- persistent first/last mel tiles (f32) -------
            W_FIRST = NGS[0] * CH + 6
            W_LAST = NGS[-1] * CH + 6
            T0_LAST = (NCH - NGS[-1]) * CH
            LASTREAL = T - (T0_LAST - 3)
            melt_firsts, melt_lasts = [], []
            for bb in range(BPC):
                mf = pstat.tile([128, W_FIRST], F32R, name=f"mf{bb}")
                melt_firsts.append(mf)
                ml = pstat.tile([128, W_LAST], BF16, name=f"ml{bb}")
                melt_lasts.append(ml)

            # constants ride the sync (HWDGE) queue ahead of the first mel
            # tile: tiny bytes, and the idle SP sequencer absorbs the per-DMA
            # issue cost (GpSimd Q7 descgen is ~670ns each and would delay
            # the odd-tile mel stream; ACT must stay DMA-free for sq).
            WZb = pstat.tile([128, 257], BF16)
            nc.sync.dma_start(out=WZb, in_=wzb_d[:, :])
            WZ32 = pstat.tile([128, 257], F32R)
            nc.sync.dma_start(out=WZ32, in_=wz32_d[:, :].bitcast(F32R))

            # first mel tile DMA goes out right after the one-hot banks
            nc.sync.dma_start(
                out=melt_firsts[0][:, 3 : 3 + CH + 3],
                in_=mel[0, :, 0 : CH + 3].bitcast(F32R),
            )

            indB = pstat.tile([128, 2 * NROWS], BF16)
            SMb = pstat.tile([NROWS, 256], BF16)
            gate01 = pstat.tile([NROWS, 256], BF16)

            def emit_consts_late():
                nc.sync.dma_start(out=indB, in_=indB_d[:, :])
                nc.sync.dma_start(out=SMb, in_=SM_d[:, :])
                nc.sync.dma_start(out=gate01, in_=gate_d[:, :])

            # dummy Sqrt: forces the sqrt_and_others table
            # (square/abs/copy/identity/sqrt) once, at t~0 - no mid-kernel
            # ACT table reload.  Square (the sq op) lives in the same set.
            dummy = pstat.tile([1, 1], F32)
            nc.vector.memset(dummy, 1.0)
            nc.scalar.activation(out=dummy, in_=dummy, func=ACTF.Sqrt)
            epsT = pstat.tile([128, 1], F32)
            nc.vector.memset(epsT, float(EPS2))

            # zero the persistent tile pads (f32 views)
            for bb in range(BPC):
                nc.vector.memset(melt_firsts[bb][:, 0:3].bitcast(F32), 0.0)
                nc.vector.memset(melt_lasts[bb][:, LASTREAL:W_LAST], 0.0)

            r = pstat.tile([NROWS, 256], F32)
            t1sv = pstat.tile([NROWS, 260], BF16)
            g05b = pstat.tile([NROWS, 256], BF16)
            rU = pstat.tile([NROWS, 256], F32)
            rDA = pstat.tile([NROWS, 256], F32)
            rD0_f = pstat.tile([NROWS, 256], F32)

            def emit_smalls():
                nc.sync.dma_start(out=r, in_=rpk_d[:, :])
                nc.sync.dma_start(out=t1sv, in_=t1sv_d[:, :])
                nc.sync.dma_start(out=g05b, in_=g05_d[:, :])
                nc.sync.dma_start(out=rU, in_=rU_d[:, :])
                nc.sync.dma_start(out=rDA, in_=rDA_d[:, :])
                nc.sync.dma_start(out=rD0_f, in_=rD0_d[:, :])

            # ---------------- PSUM ----------------
            # psS_b  [128, 512]: col sums,    S[rg, h], h -> t = 256rg-2+h
            # psQD_b [128,1024]: bank0 Q[rg, 0:261] (t=256rg-3+h),
            #                    bank1 D[rg, 0:260] at cols 512:772
            psS = [pps.tile([128, 512], F32, name=f"psS{b}") for b in range(BPC)]
            psQD = [pps.tile([128, 1024], F32, name=f"psQD{b}") for b in range(BPC)]
            # two extra banks hold the tiny temporal/broadcast outputs
            psTB = pps2.tile([128, 512], F32)
            psBB = pps2.tile([128, 512], F32)

            def _mm(ti, b, row_local, bank, lhsT_onehot, cols, rhs):
                rg = SBASE[b] + row_local
                lhsT = lhsT_onehot[:, 128 - rg : 256 - rg]
                nc.tensor.matmul(
                    out=bank[0:128, cols], lhsT=lhsT, rhs=rhs,
                    start=(row_local == 0),
                    stop=(row_local == NCH - 1),
                )

            def emit_s_mm(b, row_local, melt, c0, cast=False):
                _mm(0, b, row_local, psS[b], WZb if cast else WZ32,
                    slice(0, 260), melt[:, c0 + 1 : c0 + 261])

            def emit_qd_mm(b, row_local, qd, c0):
                _mm(1, b, row_local, psQD[b], WZb, slice(0, 261),
                    qd[:, c0 : c0 + 261])
                _mm(2, b, row_local, psQD[b], WZb, slice(512, 772),
                    qd[:, CA + c0 : CA + c0 + 260])

            def emit_tile(b, j, g0, ng):
                t0 = g0 * CH
                wmel = ng * CH + 6
                src_lo = t0 - 3
                src_hi = min(T, t0 + ng * CH + 3)
                cast = (j % 2 == 1)
                if j == 0:
                    melt = melt_firsts[b]
                elif j == len(NGS) - 1:
                    melt = melt_lasts[b]
                elif cast:
                    melt = pmelc.tile([128, wmel], BF16, tag="meltc")
                else:
                    melt = pmelf.tile([128, wmel], F32R, tag="melt")
                lo_pad = 3 if j == 0 else 0
                s_lo = src_lo + lo_pad
                w_real = src_hi - s_lo
                if not (b == 0 and j == 0):  # (0,0) already issued up top
                    eng = nc.sync if (j % 2 == 0) else nc.gpsimd
                    eng.dma_start(
                        out=melt[:, lo_pad : lo_pad + w_real],
                        in_=mel[b, :, s_lo:src_hi]
                        if cast
                        else mel[b, :, s_lo:src_hi].bitcast(F32R),
                    )
                # S matmuls run straight off the f32 data (f32r stream);
                # cast tiles stream bf16 with the bf16 one-hot bank
                for k in range(ng):
                    emit_s_mm(b, g0 + k, melt, k * CH, cast=cast)
                # combined sq|cross buffer (bf16): sq at [0,wmel),
                # cross at [CA, CA+wmel-1)
                qd = pqd.tile([128, QDW], BF16, tag="qd")
                wx = wmel - 1
                if cast:
                    sq_in = melt[:, 0:wmel]
                    c0v, c1v = melt[:, 0:wx], melt[:, 1 : wx + 1]
                else:
                    sq_in = melt[:, 0:wmel].bitcast(F32)
                    c0v = melt[:, 0:wx].bitcast(F32)
                    c1v = melt[:, 1 : wx + 1].bitcast(F32)
                nc.scalar.activation(out=qd[:, 0:wmel], in_=sq_in,
                                     func=ACTF.Square)
                nc.vector.tensor_tensor(
                    out=qd[:, CA : CA + wx], in0=c0v, in1=c1v, op=ALU.mult,
                )
                for k in range(ng):
                    emit_qd_mm(b, g0 + k, qd, k * CH)

            # ---------------- per-sample tail (list of closures) ----------
            def prep_tail(b):
                # scratch tiles whose out-of-sample rows must be zero: memset
                # them mid-stream on the idle Q7 so the serial tail chain
                # doesn't pay for it
                pre = {}
                pre["W"] = pstat.tile([128, 256], BF16, name=f"W{b}")
                nc.vector.memset(pre["W"], 0.0)
                pre["Wsq"] = pstat.tile([128, 256], BF16, name=f"Wq{b}")
                nc.vector.memset(pre["Wsq"], 0.0)
                pre["sd"] = pstat.tile([NROWS, 260], F32, name=f"sd{b}")
                nc.vector.memset(pre["sd"], 1.0)
                for nm in ("mU", "mDA", "mD0"):
                    pre[nm] = pstat.tile([NROWS, 256], F32, name=f"{nm}{b}")
                    nc.vector.memset(pre[nm], 0.0)
                return pre

            def make_tail(b, pre):
                r0 = SBASE[b]
                sl = slice(r0, r0 + NCH)
                st = dict(pre)

                def p1():
                    # temporal: Sb=bf16(psS), W=win5(Sb)
                    st["Sb"] = pstat.tile([NROWS, 260], BF16, name=f"Sb{b}")
                    nc.scalar.activation(
                        out=st["Sb"][sl], in_=psS[b][sl, 0:260], func=ACTF.Copy
                    )
                    wa = pstat.tile([NROWS, 258], BF16, name=f"wa{b}")
                    nc.vector.tensor_tensor(
                        out=wa[sl], in0=st["Sb"][sl][:, 0:258],
                        in1=st["Sb"][sl][:, 1:259], op=ALU.add,
                    )
                    nc.vector.tensor_tensor(
                        out=wa[sl][:, 0:256], in0=wa[sl][:, 0:256],
                        in1=wa[sl][:, 2:258], op=ALU.add,
                    )
                    nc.vector.tensor_tensor(
                        out=st["W"][sl], in0=wa[sl][:, 0:256],
                        in1=st["Sb"][sl][:, 4:260], op=ALU.add,
                    )
                    # cos chain start: Q to SBUF (bf16), den2 = Qa*Qb
                    st["Qs"] = pstat.tile([NROWS, 261], BF16, name=f"Qs{b}")
                    nc.scalar.activation(
                        out=st["Qs"][sl], in_=psQD[b][sl, 0:261], func=ACTF.Copy
                    )
                    st["den2"] = pstat.tile([NROWS, 260], BF16, name=f"dn{b}")
                    nc.vector.tensor_tensor(
                        out=st["den2"][sl], in0=st["Qs"][sl][:, 0:260],
                        in1=st["Qs"][sl][:, 1:261], op=ALU.mult,
                    )
                    nc.scalar.activation(
                        out=st["sd"][sl], in_=st["den2"][sl], func=ACTF.Sqrt,
                        bias=epsT[sl],
                    )

                def p2():
                    # custom-DVE ops misbehave at partition base 64 on HW:
                    # run the reciprocal over the full [0:NROWS] span (base 0)
                    st["rs"] = pstat.tile([NROWS, 260], F32, name=f"rs{b}")
                    nc.vector.reciprocal_approx_fast(
                        out=st["rs"][0:NROWS, :], in_=st["sd"][0:NROWS, :]
                    )
                    st["cos"] = pstat.tile([NROWS, 260], BF16, name=f"cs{b}")
                    nc.vector.tensor_tensor(
                        out=st["cos"][sl], in0=psQD[b][sl, 512:772],
                        in1=st["rs"][sl], op=ALU.mult,
                    )
                    st["consH"] = pstat.tile([NROWS, 260], BF16, name=f"ch{b}")
                    nc.vector.scalar_tensor_tensor(
                        out=st["consH"][sl], in0=st["cos"][sl], scalar=float(w0),
                        in1=t1sv[sl], op0=ALU.mult, op1=ALU.add,
                    )
                    # temporal sums via PE: block-indicator lhsT lands the
                    # per-sample sums broadcast onto that sample's partitions
                    nc.scalar.activation(
                        out=st["Wsq"][sl], in_=st["W"][sl], func=ACTF.Square
                    )
                    bank = psTB if b == 0 else psBB
                    lT = indB[:, b * NROWS : (b + 1) * NROWS]
                    nc.tensor.matmul(
                        out=bank[0:NROWS, 0:256], lhsT=lT,
                        rhs=st["W"][0:128, 0:256], start=True, stop=True,
                    )
                    nc.tensor.matmul(
                        out=bank[0:NROWS, 256:512], lhsT=lT,
                        rhs=st["Wsq"][0:128, 0:256], start=True, stop=True,
                    )

                def p3():
                    # scalar std/threshold chain on this sample's partitions
                    # (psT rows carry the sample sums broadcast per partition)
                    bank = psTB if b == 0 else psBB
                    sx = pstat.tile([128, 2], F32, name=f"sx{b}")
                    scr = pstat.tile([NROWS, 256], F32, name=f"scr{b}")
                    nc.scalar.activation(
                        out=scr[sl], in_=bank[sl, 0:256], func=ACTF.Copy,
                        accum_out=sx[sl, 0:1],
                    )
                    nc.scalar.activation(
                        out=scr[sl], in_=bank[sl, 256:512], func=ACTF.Copy,
                        accum_out=sx[sl, 1:2],
                    )
                    nc.vector.tensor_scalar_mul(
                        out=sx[sl, 0:1], in0=sx[sl, 0:1], scalar1=float(SMSC)
                    )
                    nc.vector.tensor_scalar_mul(
                        out=sx[sl, 1:2], in0=sx[sl, 1:2], scalar1=float(SMSC * SMSC)
                    )
                    sc = pstat.tile([128, 4], F32, name=f"sc{b}")
                    s2 = sc[sl, 0:1]
                    nc.vector.tensor_tensor(
                        out=s2, in0=sx[sl, 0:1], in1=sx[sl, 0:1], op=ALU.mult
                    )
                    nc.vector.tensor_scalar_mul(out=s2, in0=s2, scalar1=1.0 / float(T))
                    var = sc[sl, 1:2]
                    nc.vector.tensor_tensor(
                        out=var, in0=sx[sl, 1:2], in1=s2, op=ALU.subtract
                    )
                    nc.vector.tensor_scalar_mul(
                        out=var, in0=var, scalar1=1.0 / float(T - 1)
                    )
                    std = sc[sl, 2:3]
                    nc.scalar.activation(out=std, in_=var, func=ACTF.Sqrt)
                    # w2t = w2*(1-std); thrH = 0.7-w2t, thrL = 0.4-w2t
                    w2t = sc[sl, 3:4]
                    nc.vector.tensor_scalar(
                        out=w2t, in0=std, scalar1=-1.0, scalar2=1.0,
                        op0=ALU.mult, op1=ALU.add,
                    )
                    nc.vector.tensor_scalar_mul(out=w2t, in0=w2t, scalar1=float(w2))
                    st["thrS"] = pstat.tile([128, 2], F32, name=f"th{b}")
                    nc.vector.tensor_scalar(
                        out=st["thrS"][sl, 0:1], in0=w2t, scalar1=-1.0, scalar2=0.7,
                        op0=ALU.mult, op1=ALU.add,
                    )
                    nc.vector.tensor_scalar(
                        out=st["thrS"][sl, 1:2], in0=w2t, scalar1=-1.0, scalar2=0.4,
                        op0=ALU.mult, op1=ALU.add,
                    )
                    # local-mean chain
                    st["w5"] = pstat.tile([NROWS, 256], BF16, name=f"w5{b}")
                    ca = pstat.tile([NROWS, 258], BF16, name=f"ca{b}")
                    nc.vector.tensor_tensor(
                        out=ca[sl], in0=st["consH"][sl][:, 0:258],
                        in1=st["consH"][sl][:, 1:259], op=ALU.add,
                    )
                    nc.vector.tensor_tensor(
                        out=ca[sl][:, 0:256], in0=ca[sl][:, 0:256],
                        in1=ca[sl][:, 2:258], op=ALU.add,
                    )
                    nc.vector.tensor_tensor(
                        out=st["w5"][sl], in0=ca[sl][:, 0:256],
                        in1=st["consH"][sl][:, 4:260], op=ALU.add,
                    )
                    st["local"] = pstat.tile([NROWS, 256], BF16, name=f"lc{b}")
                    nc.vector.tensor_tensor(
                        out=st["local"][sl], in0=st["w5"][sl], in1=SMb[sl],
                        op=ALU.mult,
                    )

                def p4():
                    # grads branch
                    st["gr"] = pstat.tile([NROWS, 256], BF16, name=f"gr{b}")
                    nc.vector.tensor_tensor(
                        out=st["gr"][sl], in0=st["consH"][sl][:, 2:258],
                        in1=st["consH"][sl][:, 1:257], op=ALU.subtract,
                    )
                    st["gsq"] = pstat.tile([NROWS, 256], BF16, name=f"gq{b}")
                    nc.vector.tensor_tensor(
                        out=st["gsq"][sl], in0=st["gr"][sl], in1=st["gr"][sl],
                        op=ALU.mult,
                    )
                    st["A"] = pstat.tile([NROWS, 256], BF16, name=f"A{b}")
                    nc.vector.tensor_scalar(
                        out=st["A"][sl], in0=st["gsq"][sl], scalar1=th2,
                        scalar2=None, op0=ALU.is_gt,
                    )
                    # compares
                    st["u"] = pstat.tile([NROWS, 256], BF16, name=f"u{b}")
                    nc.vector.tensor_scalar(
                        out=st["u"][sl], in0=st["local"][sl],
                        scalar1=st["thrS"][sl][:, 0:1], scalar2=None, op0=ALU.is_gt,
                    )
                    st["v"] = pstat.tile([NROWS, 256], BF16, name=f"v{b}")
                    nc.vector.tensor_scalar(
                        out=st["v"][sl], in0=st["local"][sl],
                        scalar1=st["thrS"][sl][:, 1:2], scalar2=None, op0=ALU.is_lt,
                    )

                def p4b():
                    if not KDBG:
                        return
                    nc.sync.dma_start(out=dbg["dbg_Sb"][b], in_=st["Sb"][0:NROWS, 0:260])
                    nc.sync.dma_start(out=dbg["dbg_W"][b], in_=st["W"][0:NROWS, 0:256])
                    nc.sync.dma_start(out=dbg["dbg_thr"][b], in_=st["thrS"][0:NROWS, 0:2])
                    nc.sync.dma_start(
                        out=dbg["dbg_local"][b], in_=st["local"][0:NROWS, 0:256]
                    )
                    nc.sync.dma_start(
                        out=dbg["dbg_consH"][b], in_=st["consH"][0:NROWS, 0:260]
                    )
                    nc.sync.dma_start(out=dbg["dbg_A"][b], in_=st["A"][0:NROWS, 0:256])
                    nc.sync.dma_start(out=dbg["dbg_Qs"][b], in_=st["Qs"][0:NROWS, 0:261])
                    nc.sync.dma_start(out=dbg["dbg_cos"][b], in_=st["cos"][0:NROWS, 0:260])
                    nc.sync.dma_start(out=dbg["dbg_t1sv"][b], in_=t1sv[0:NROWS, 0:260])
                    nc.sync.dma_start(
                        out=dbg["dbg_den2"][b], in_=st["den2"][0:NROWS, 0:260]
                    )
                    nc.sync.dma_start(out=dbg["dbg_sd"][b], in_=st["sd"][0:NROWS, 0:260])
                    nc.sync.dma_start(out=dbg["dbg_rs"][b], in_=st["rs"][0:NROWS, 0:260])
                    Dsd = pstat.tile([NROWS, 260], F32, name=f"Dsd{b}")
                    nc.scalar.activation(
                        out=Dsd[sl], in_=psQD[b][sl, 512:772], func=ACTF.Copy
                    )
                    nc.sync.dma_start(out=dbg["dbg_Ds"][b], in_=Dsd[0:NROWS, 0:260])

                def p5():
                    # up = v&gate, dn = u&gate ; act0 = g05|A ; masks f32
                    st["up"] = pstat.tile([NROWS, 256], BF16, name=f"up{b}")
                    nc.vector.tensor_tensor(
                        out=st["up"][sl], in0=st["v"][sl], in1=gate01[sl],
                        op=ALU.mult,
                    )
                    st["dn"] = pstat.tile([NROWS, 256], BF16, name=f"dnm{b}")
                    nc.vector.tensor_tensor(
                        out=st["dn"][sl], in0=st["u"][sl], in1=gate01[sl],
                        op=ALU.mult,
                    )
                    st["act0"] = pstat.tile([NROWS, 256], BF16, name=f"a0{b}")
                    nc.vector.tensor_tensor(
                        out=st["act0"][sl], in0=g05b[sl], in1=st["A"][sl],
                        op=ALU.max,
                    )
                    st["nA"] = pstat.tile([NROWS, 256], BF16, name=f"nA{b}")
                    nc.vector.tensor_scalar(
                        out=st["nA"][sl], in0=st["A"][sl], scalar1=-1.0,
                        scalar2=1.0, op0=ALU.mult, op1=ALU.add,
                    )
                    st["ng"] = pstat.tile([NROWS, 256], BF16, name=f"ng{b}")
                    nc.vector.tensor_tensor(
                        out=st["ng"][sl], in0=st["nA"][sl], in1=g05b[sl],
                        op=ALU.mult,
                    )

                def p6():
                    # copy_predicated is also run full-span at base 0 (custom
                    # partition-sensitive op); masks are zeroed outside [sl]
                    mU, mDA, mD0 = st["mU"], st["mDA"], st["mD0"]
                    nc.vector.tensor_tensor(
                        out=mU[sl], in0=st["up"][sl], in1=st["act0"][sl],
                        op=ALU.mult,
                    )
                    nc.vector.tensor_tensor(
                        out=mDA[sl], in0=st["dn"][sl], in1=st["A"][sl],
                        op=ALU.mult,
                    )
                    nc.vector.tensor_tensor(
                        out=mD0[sl], in0=st["dn"][sl], in1=st["ng"][sl],
                        op=ALU.mult,
                    )
                    fs = slice(0, NROWS)
                    nc.vector.copy_predicated(
                        out=r[fs], mask=mU[fs].bitcast(mybir.dt.int32), data=rU[fs]
                    )
                    nc.vector.copy_predicated(
                        out=r[fs], mask=mDA[fs].bitcast(mybir.dt.int32), data=rDA[fs]
                    )
                    nc.vector.copy_predicated(
                        out=r[fs], mask=mD0[fs].bitcast(mybir.dt.int32),
                        data=rD0_f[fs],
                    )

                def p7():
                    ob = out[b]
                    eng = nc.scalar if b == 0 else nc.sync
                    eng.dma_start(
                        out=bass.AP(
                            tensor=ob.tensor, offset=ob.offset,
                            ap=[[256, 46], [1, 256]],
                        ),
                        in_=r[r0 : r0 + 46, :],
                    )
                    eng.dma_start(
                        out=bass.AP(
                            tensor=ob.tensor, offset=ob.offset + 256 * 46,
                            ap=[[256, 1], [1, LASTW]],
                        ),
                        in_=r[r0 + 46 : r0 + 47, 0:LASTW],
                    )

                return [p1, p2, p3, p4, p4b, p5, p6, p7]

            # ---------------- emission schedule ----------------
            # stream sample 0, with spec/r precompute interleaved
            g0 = 0
            for j, ng in enumerate(NGS):
                emit_tile(0, j, g0, ng)
                g0 += ng

            emit_smalls()
            emit_consts_late()
            pre0 = prep_tail(0)
            pre1 = prep_tail(1)

            tail0 = make_tail(0, pre0)
            # stream sample 1 with tail-0 pieces interleaved
            g0 = 0
            for j, ng in enumerate(NGS):
                emit_tile(1, j, g0, ng)
                g0 += ng
                if j >= 1 and tail0:
                    tail0.pop(0)()
            while tail0:
                tail0.pop(0)()

            for p in make_tail(1, pre1):
                p()

    nc.compile()
    return nc


_CACHE = {}


def _get_nc(wbytes):
    if wbytes not in _CACHE:
        w = np.frombuffer(wbytes, np.float32)
        _CACHE[wbytes] = build_nc(float(w[0]), float(w[1]), float(w[2]))
    return _CACHE[wbytes]


def kernel(**inputs):
    mel = np.ascontiguousarray(np.asarray(inputs["mel_features"], np.float32))
    spec = np.ascontiguousarray(np.asarray(inputs["spectral_features"], np.float32))
    init = np.ascontiguousarray(np.asarray(inputs["initial_boundaries"], np.float32))
    sw = np.asarray(inputs["similarity_weights"], np.float32)
    w = _softmax_f32(sw)
    nc = _get_nc(w.tobytes())

    import ml_dtypes

    bf = ml_dtypes.bfloat16
    in_maps = []
    for c in range(NCORES):
        s = slice(c * BPC, (c + 1) * BPC)
        rpk, t1, g05, rUh, rDAh, rD0h = _host_pack(spec[s], init[s], w[1])
        in_maps.append(
            {
                "mel_features": np.ascontiguousarray(mel[s]),
                "rpk": rpk,
                "t1svh": t1.astype(bf),
                "g05h": g05.astype(bf),
                "rUh": rUh,
                "rDAh": rDAh,
                "rD0h": rD0h,
            }
        )
    res = run_bass_kernel_spmd(nc, in_maps, core_ids=list(range(NCORES)))
    global _LAST_RESULT
    _LAST_RESULT = res
    outs = [np.asarray(res.results[c]["out"], np.float32) for c in range(NCORES)]
    return np.concatenate(outs, axis=0)


_LAST_RESULT = None


if __name__ == "__main__":
    nc = build_nc(1 / 3, 1 / 3, 1 / 3)
    ninst = sum(len(b.instructions) for b in nc.m.functions[0].blocks)
    print("built ok, instructions:", ninst)


# revision 25
# speedup vs baseline: 1.0594x; 1.0594x over previous
"""Trainium2 Bass kernel for nn_AdaptiveBoundaryRefinement_45861660787095.

Self-contained: takes FULL inputs (B=16,M=128,T=12000), shards batch across 8
NeuronCores (2 samples/core), runs a Bass/Tile kernel per core, gathers.

v3 design (v2 tail numerics kept verbatim; front-end rebuilt from trace data):
- mel streams as RAW F32 over two DMA queues (sync HWDGE + gpsimd SWDGE,
  alternating tiles).  The v2 f32->bf16 cast-DMA saturated all 16 SDMA
  engines at only ~280 GB/s read-side; plain f32 streams at the ~358 GB/s
  HBM ceiling and frees the cast entirely.
- S (column sums) matmuls stream the f32 data directly as f32r: with moving
  dim >=256 the PE streams f32r at 1 cycle/row, so no bf16 copy of mel is
  needed anywhere.
- sq = mel^2 moves to the Scalar engine (ACT Square, f32-in/bf16-out,
  1 elem/lane/cycle) - the DVE was the measured critical resource (50.5us
  busy, 54% occ) and TT tops out at 2x regardless of dtype.
- cross = mel[t]*mel[t+1] stays on DVE (f32 1x) except a few big tiles that
  run on GpSimd, balancing all three elementwise engines under the stream.
- Q and D fuse into ONE matmul per chunk: sq and cross live in one combined
  SBUF buffer (cross at column offset CA) and the rhs is a 2-block strided
  AP [[CA,2],[1,261]]; the out AP [[512,2],[1,261]] writes the Q bank and D
  bank of a 2-bank PSUM tile in one pass.  Halves the MM count -> fewer
  fixed overheads and a denser PE stream (keeps the p-state high).
- Per-sample tail (win5 sums, cos, thresholds, closed-form 5-iteration
  refinement via predicated copies) is byte-identical to the validated v2.
- The batch-global early-stop of the reference is a mathematical no-op.
"""

import os
import sys

import numpy as np

_TRN_REPO = "/opt/trn_rl_repo"
if _TRN_REPO not in sys.path:
    sys.path.insert(0, _TRN_REPO)

import concourse.bass as bass
import concourse.bacc as bacc
import concourse.mybir as mybir
import concourse.tile as tile
from concourse.bass_utils import run_bass_kernel_spmd

F32 = mybir.dt.float32
F32R = mybir.dt.float32r
BF16 = mybir.dt.bfloat16
ALU = mybir.AluOpType
ACTF = mybir.ActivationFunctionType
AX = mybir.AxisListType

B, M, T = 16, 128, 12000
NCORES = 8
BPC = B // NCORES            # samples per core = 2
CH = 256                     # chunk width
NCH = (T + CH - 1) // CH     # 47 chunks per sample
EPS2 = 1e-12
GRAD_THRESH = 0.15
LASTW = T - CH * (NCH - 1)   # 224 real cols in the last chunk
NGS = [1, 4, 8, 8, 8, 8, 8, 2]  # chunks per mel tile (small ramp/tail tiles)
SBASE = (0, 64)              # partition base of each sample's chunk rows
NROWS = SBASE[1] + NCH       # 111
CA = 2056                    # cross region offset inside the combined qd tile
QDW = CA + 2053 + 3          # combined sq|cross tile width (bf16)
# odd-j tiles ride the gpsimd (SWDGE) queue and are cast f32->bf16 during the
# DMA: their cross runs at DVE 2x and their S-matmul streams bf16.  Even-j
# tiles stay f32 on the sync (HWDGE) queue at full HBM rate.


def _softmax_f32(x):
    x = np.asarray(x, np.float32)
    m = np.max(x).astype(np.float32)
    e = np.exp((x - m).astype(np.float32)).astype(np.float32)
    return (e / e.sum(dtype=np.float32).astype(np.float32)).astype(np.float32)


def _const_masks(w0, w1):
    import ml_dtypes

    bf = ml_dtypes.bfloat16
    # w1vC [NROWS, 260]: w1 where col maps to t in [0, T), else 0
    # (col h of chunk p -> t = 256p - 2 + h).  Also t=0 keeps w1 (spec_sim=1
    # there via the specH shift trick).
    w1vC = np.full((NROWS, 260), np.float32(w1), np.float32)
    for r0 in SBASE:
        w1vC[r0, 0:2] = 0.0                       # t < 0
        w1vC[r0 + NCH - 1, LASTW + 2 : 260] = 0.0  # t >= T
    # SMb [NROWS, 256]: 0.2 interior, 0.25 at t=1 and t=T-2 (win4 edges)
    SM = np.full((NROWS, 256), np.float32(0.2), np.float32)
    for r0 in SBASE:
        SM[r0, 1] = 0.25
        SM[r0 + NCH - 1, LASTW - 2] = 0.25
    # gate01 [NROWS, 256]: 1 for t in [1, T-2], else 0
    gate = np.ones((NROWS, 256), np.float32)
    for r0 in SBASE:
        gate[r0, 0] = 0.0
        gate[r0 + NCH - 1, LASTW - 1 : 256] = 0.0
    # wzb one-hot bank [128, 257]: col 128 is ones; slice [128-j : 256-j]
    # puts the hot column at position j (j in 0..127).
    wzb = np.zeros((128, 257), bf)
    wzb[:, 128] = 1.0
    wz32 = np.zeros((128, 257), np.float32)
    wz32[:, 128] = 1.0
    # indB [128, 2*NROWS]: cols [0:NROWS] sample-0 block-indicator lhsT,
    # cols [NROWS:2*NROWS] sample-1.  out[i,:] = sum_{p in sample} W[p,:]
    # lands on every row i of that sample -> per-partition broadcast sums.
    indB = np.zeros((128, 2 * NROWS), bf)
    indB[0:NCH, 0:NCH] = 1.0
    indB[SBASE[1] : SBASE[1] + NCH, NROWS + SBASE[1] : NROWS + SBASE[1] + NCH] = 1.0
    return w1vC.astype(bf), SM.astype(bf), gate.astype(bf), wzb, indB, wz32


def _host_pack(spec2, init2, w1):
    """Row-pack the [BPC,T] side inputs into the kernel's [NROWS,*] layouts.

    All arithmetic in f32 to mirror the device ops this replaces."""
    rpk = np.zeros((NROWS, 256), np.float32)
    t1 = np.zeros((NROWS, 260), np.float32)
    g05 = np.zeros((NROWS, 256), np.float32)
    rU = np.zeros((NROWS, 256), np.float32)
    rDA = np.zeros((NROWS, 256), np.float32)
    rD0 = np.zeros((NROWS, 256), np.float32)
    for b in range(BPC):
        r0 = SBASE[b]
        full = np.zeros(NCH * 256, np.float32)
        full[:T] = init2[b]
        R = full.reshape(NCH, 256)
        rpk[r0 : r0 + NCH] = R
        g05[r0 : r0 + NCH] = (R > np.float32(0.5)).astype(np.float32)
        rU[r0 : r0 + NCH] = np.minimum(R + np.float32(0.5), np.float32(1.0))
        rDA[r0 : r0 + NCH] = np.maximum(R - np.float32(0.5), np.float32(0.0))
        y = R * np.float32(10.0) - np.float32(5.0)
        k = np.zeros_like(R)
        for j in range(5):
            k += (y > np.float32(j)).astype(np.float32)
        rD0[r0 : r0 + NCH] = R + np.float32(-0.1) * k
        # t1sv: col h of chunk row rl -> t = 256*rl - 2 + h; w1/(1+|dspec|),
        # spec_sim(0)=1, zero outside [0, T)
        sp = spec2[b]
        ssim = np.ones(T, np.float32)
        ssim[1:] = np.float32(1.0) / (
            np.float32(1.0) + np.abs(sp[1:] - sp[:-1]).astype(np.float32)
        )
        tv = np.zeros(NCH * 256 + 260, np.float32)
        tv[2 : 2 + T] = np.float32(w1) * ssim
        for rl in range(NCH):
            t1[r0 + rl] = tv[256 * rl : 256 * rl + 260]
    return rpk, t1, g05, rU, rDA, rD0


def build_nc(w0, w1, w2):
    nc = bacc.Bacc("TRN2", target_bir_lowering=False, debug=False)
    mel = nc.dram_tensor("mel_features", [BPC, M, T], F32, kind="ExternalInput")
    # host-precomputed, row-packed [NROWS, *] side inputs (see _host_pack)
    rpk_d = nc.dram_tensor("rpk", [NROWS, 256], F32, kind="ExternalInput")
    t1sv_d = nc.dram_tensor("t1svh", [NROWS, 260], BF16, kind="ExternalInput")
    g05_d = nc.dram_tensor("g05h", [NROWS, 256], BF16, kind="ExternalInput")
    rU_d = nc.dram_tensor("rUh", [NROWS, 256], F32, kind="ExternalInput")
    rDA_d = nc.dram_tensor("rDAh", [NROWS, 256], F32, kind="ExternalInput")
    rD0_d = nc.dram_tensor("rD0h", [NROWS, 256], F32, kind="ExternalInput")
    out = nc.dram_tensor("out", [BPC, T], F32, kind="ExternalOutput")
    KDBG = bool(os.environ.get("KDBG"))
    dbg = {}
    if KDBG:
        for nm, w, dt in (
            ("dbg_W", 256, BF16), ("dbg_thr", 2, F32), ("dbg_local", 256, BF16),
            ("dbg_consH", 260, BF16), ("dbg_A", 256, BF16), ("dbg_Sb", 260, BF16),
            ("dbg_Qs", 261, BF16), ("dbg_cos", 260, BF16), ("dbg_t1sv", 260, BF16),
            ("dbg_Ds", 260, F32), ("dbg_den2", 260, BF16), ("dbg_sd", 260, F32),
            ("dbg_rs", 260, F32),
        ):
            dbg[nm] = nc.dram_tensor(nm, [BPC, NROWS, w], dt, kind="ExternalOutput")

    _w1vC_np, SM_np, gate_np, wzb_np, indB_np, wz32_np = _const_masks(w0, w1)
    wzb_d = nc.inline_tensor(wzb_np, name="wzbc")
    wz32_d = nc.inline_tensor(wz32_np, name="wz32c")
    indB_d = nc.inline_tensor(indB_np, name="indB")
    SM_d = nc.inline_tensor(SM_np, name="SMb")
    gate_d = nc.inline_tensor(gate_np, name="gate01")

    th2 = float(np.float32(GRAD_THRESH) * np.float32(GRAD_THRESH))
    SMSC = 0.2 / 128.0

    with tile.TileContext(nc) as tc:
        with (
            tc.tile_pool(name="qd", bufs=6) as pqd,
            tc.tile_pool(name="stat", bufs=1) as pstat,
            tc.tile_pool(name="ps", bufs=1, space="PSUM") as pps,
            tc.tile_pool(name="ps2", bufs=1, space="PSUM") as pps2,
        ):
            # ---------------- persistent mel tiles (all 16 fit in SBUF) ---
            W_LAST = NGS[-1] * CH + 6
            T0_LAST = (NCH - NGS[-1]) * CH
            LASTREAL = T - (T0_LAST - 3)
            melts = []  # [b][j]
            for bb in range(BPC):
                row = []
                for jj, ngj in enumerate(NGS):
                    wj = ngj * CH + 6
                    dt = F32R if jj % 2 == 0 else BF16
                    row.append(pstat.tile([128, wj], dt, name=f"mel{bb}_{jj}"))
                melts.append(row)

            # constants ride the sync (HWDGE) queue ahead of the first mel
            # tile: tiny bytes, and the idle SP sequencer absorbs the per-DMA
            # issue cost (GpSimd Q7 descgen is ~670ns each and would delay
            # the odd-tile mel stream; ACT must stay DMA-free for sq).
            WZb = pstat.tile([128, 257], BF16)
            nc.sync.dma_start(out=WZb, in_=wzb_d[:, :])
            WZ32 = pstat.tile([128, 257], F32R)
            nc.sync.dma_start(out=WZ32, in_=wz32_d[:, :].bitcast(F32R))

            # all 16 mel tile DMAs issue up-front, in ring order per queue:
            # sync (HWDGE) carries the even f32 tiles, gpsimd (SWDGE) the odd
            # cast-to-bf16 tiles.  No pool recycling -> no issue stalls.
            def _mel_dma(b, j):
                ng = NGS[j]
                g0 = sum(NGS[:j])
                t0 = g0 * CH
                src_lo = t0 - 3
                src_hi = min(T, t0 + ng * CH + 3)
                melt = melts[b][j]
                lo_pad = 3 if j == 0 else 0
                s_lo = src_lo + lo_pad
                w_real = src_hi - s_lo
                if j % 2 == 0:
                    nc.sync.dma_start(
                        out=melt[:, lo_pad : lo_pad + w_real],
                        in_=mel[b, :, s_lo:src_hi].bitcast(F32R),
                    )
                else:
                    nc.gpsimd.dma_start(
                        out=melt[:, lo_pad : lo_pad + w_real],
                        in_=mel[b, :, s_lo:src_hi],
                    )

            for bb in range(BPC):
                for jj in range(0, len(NGS), 2):
                    _mel_dma(bb, jj)
            for bb in range(BPC):
                for jj in range(1, len(NGS), 2):
                    _mel_dma(bb, jj)

            indB = pstat.tile([128, 2 * NROWS], BF16)
            SMb = pstat.tile([NROWS, 256], BF16)
            gate01 = pstat.tile([NROWS, 256], BF16)

            def emit_consts_late():
                nc.sync.dma_start(out=indB, in_=indB_d[:, :])
                nc.sync.dma_start(out=SMb, in_=SM_d[:, :])
                nc.sync.dma_start(out=gate01, in_=gate_d[:, :])

            # dummy Sqrt: forces the sqrt_and_others table
            # (square/abs/copy/identity/sqrt) once, at t~0 - no mid-kernel
            # ACT table reload.  Square (the sq op) lives in the same set.
            dummy = pstat.tile([1, 1], F32)
            nc.vector.memset(dummy, 1.0)
            nc.scalar.activation(out=dummy, in_=dummy, func=ACTF.Sqrt)
            epsT = pstat.tile([128, 1], F32)
            nc.vector.memset(epsT, float(EPS2))

            # zero the persistent first/last tile pads
            for bb in range(BPC):
                nc.vector.memset(melts[bb][0][:, 0:3].bitcast(F32), 0.0)
                nc.vector.memset(melts[bb][-1][:, LASTREAL:W_LAST], 0.0)

            r = pstat.tile([NROWS, 256], F32)
            t1sv = pstat.tile([NROWS, 260], BF16)
            g05b = pstat.tile([NROWS, 256], BF16)
            rU = pstat.tile([NROWS, 256], F32)
            rDA = pstat.tile([NROWS, 256], F32)
            rD0_f = pstat.tile([NROWS, 256], F32)

            def emit_smalls():
                nc.sync.dma_start(out=r, in_=rpk_d[:, :])
                nc.sync.dma_start(out=t1sv, in_=t1sv_d[:, :])
                nc.sync.dma_start(out=g05b, in_=g05_d[:, :])
                nc.sync.dma_start(out=rU, in_=rU_d[:, :])
                nc.sync.dma_start(out=rDA, in_=rDA_d[:, :])
                nc.sync.dma_start(out=rD0_f, in_=rD0_d[:, :])

            # ---------------- PSUM ----------------
            # psS_b  [128, 512]: col sums,    S[rg, h], h -> t = 256rg-2+h
            # psQD_b [128,1024]: bank0 Q[rg, 0:261] (t=256rg-3+h),
            #                    bank1 D[rg, 0:260] at cols 512:772
            psS = [pps.tile([128, 512], F32, name=f"psS{b}") for b in range(BPC)]
            psQD = [pps.tile([128, 1024], F32, name=f"psQD{b}") for b in range(BPC)]
            # two extra banks hold the tiny temporal/broadcast outputs
            psTB = pps2.tile([128, 512], F32)
            psBB = pps2.tile([128, 512], F32)

            def _mm(ti, b, row_local, bank, lhsT_onehot, cols, rhs):
                rg = SBASE[b] + row_local
                lhsT = lhsT_onehot[:, 128 - rg : 256 - rg]
                nc.tensor.matmul(
                    out=bank[0:128, cols], lhsT=lhsT, rhs=rhs,
                    start=(row_local == 0),
                    stop=(row_local == NCH - 1),
                )

            def emit_s_mm(b, row_local, melt, c0, cast=False):
                _mm(0, b, row_local, psS[b], WZb if cast else WZ32,
                    slice(0, 260), melt[:, c0 + 1 : c0 + 261])

            def emit_qd_mm(b, row_local, qd, c0):
                _mm(1, b, row_local, psQD[b], WZb, slice(0, 261),
                    qd[:, c0 : c0 + 261])
                _mm(2, b, row_local, psQD[b], WZb, slice(512, 772),
                    qd[:, CA + c0 : CA + c0 + 260])

            def emit_tile(b, j, g0, ng):
                wmel = ng * CH + 6
                cast = (j % 2 == 1)
                melt = melts[b][j]
                # S matmuls run straight off the f32 data (f32r stream);
                # cast tiles stream bf16 with the bf16 one-hot bank
                for k in range(ng):
                    emit_s_mm(b, g0 + k, melt, k * CH, cast=cast)
                # combined sq|cross buffer (bf16): sq at [0,wmel),
                # cross at [CA, CA+wmel-1)
                qd = pqd.tile([128, QDW], BF16, tag="qd")
                wx = wmel - 1
                if cast:
                    sq_in = melt[:, 0:wmel]
                    c0v, c1v = melt[:, 0:wx], melt[:, 1 : wx + 1]
                else:
                    sq_in = melt[:, 0:wmel].bitcast(F32)
                    c0v = melt[:, 0:wx].bitcast(F32)
                    c1v = melt[:, 1 : wx + 1].bitcast(F32)
                nc.scalar.activation(out=qd[:, 0:wmel], in_=sq_in,
                                     func=ACTF.Square)
                nc.vector.tensor_tensor(
                    out=qd[:, CA : CA + wx], in0=c0v, in1=c1v, op=ALU.mult,
                )
                for k in range(ng):
                    emit_qd_mm(b, g0 + k, qd, k * CH)

            # ---------------- per-sample tail (list of closures) ----------
            def prep_tail(b):
                # scratch tiles whose out-of-sample rows must be zero: memset
                # them mid-stream on the idle Q7 so the serial tail chain
                # doesn't pay for it
                pre = {}
                pre["W"] = pstat.tile([128, 256], BF16, name=f"W{b}")
                nc.vector.memset(pre["W"], 0.0)
                pre["Wsq"] = pstat.tile([128, 256], BF16, name=f"Wq{b}")
                nc.vector.memset(pre["Wsq"], 0.0)
                pre["sd"] = pstat.tile([NROWS, 260], F32, name=f"sd{b}")
                nc.vector.memset(pre["sd"], 1.0)
                for nm in ("mU", "mDA", "mD0"):
                    pre[nm] = pstat.tile([NROWS, 256], F32, name=f"{nm}{b}")
                    nc.vector.memset(pre[nm], 0.0)
                return pre

            def make_tail(b, pre):
                r0 = SBASE[b]
                sl = slice(r0, r0 + NCH)
                st = dict(pre)

                def p1():
                    # temporal: Sb=bf16(psS), W=win5(Sb)
                    st["Sb"] = pstat.tile([NROWS, 260], BF16, name=f"Sb{b}")
                    nc.scalar.activation(
                        out=st["Sb"][sl], in_=psS[b][sl, 0:260], func=ACTF.Copy
                    )
                    wa = pstat.tile([NROWS, 258], BF16, name=f"wa{b}")
                    nc.vector.tensor_tensor(
                        out=wa[sl], in0=st["Sb"][sl][:, 0:258],
                        in1=st["Sb"][sl][:, 1:259], op=ALU.add,
                    )
                    nc.vector.tensor_tensor(
                        out=wa[sl][:, 0:256], in0=wa[sl][:, 0:256],
                        in1=wa[sl][:, 2:258], op=ALU.add,
                    )
                    nc.vector.tensor_tensor(
                        out=st["W"][sl], in0=wa[sl][:, 0:256],
                        in1=st["Sb"][sl][:, 4:260], op=ALU.add,
                    )
                    # cos chain start: Q to SBUF (bf16), den2 = Qa*Qb
                    st["Qs"] = pstat.tile([NROWS, 261], BF16, name=f"Qs{b}")
                    nc.scalar.activation(
                        out=st["Qs"][sl], in_=psQD[b][sl, 0:261], func=ACTF.Copy
                    )
                    st["den2"] = pstat.tile([NROWS, 260], BF16, name=f"dn{b}")
                    nc.vector.tensor_tensor(
                        out=st["den2"][sl], in0=st["Qs"][sl][:, 0:260],
                        in1=st["Qs"][sl][:, 1:261], op=ALU.mult,
                    )
                    nc.scalar.activation(
                        out=st["sd"][sl], in_=st["den2"][sl], func=ACTF.Sqrt,
                        bias=epsT[sl],
                    )

                def p2():
                    # custom-DVE ops misbehave at partition base 64 on HW:
                    # run the reciprocal over the full [0:NROWS] span (base 0)
                    st["rs"] = pstat.tile([NROWS, 260], F32, name=f"rs{b}")
                    nc.vector.reciprocal_approx_fast(
                        out=st["rs"][0:NROWS, :], in_=st["sd"][0:NROWS, :]
                    )
                    st["cos"] = pstat.tile([NROWS, 260], BF16, name=f"cs{b}")
                    nc.vector.tensor_tensor(
                        out=st["cos"][sl], in0=psQD[b][sl, 512:772],
                        in1=st["rs"][sl], op=ALU.mult,
                    )
                    st["consH"] = pstat.tile([NROWS, 260], BF16, name=f"ch{b}")
                    nc.vector.scalar_tensor_tensor(
                        out=st["consH"][sl], in0=st["cos"][sl], scalar=float(w0),
                        in1=t1sv[sl], op0=ALU.mult, op1=ALU.add,
                    )
                    # temporal sums via PE: block-indicator lhsT lands the
                    # per-sample sums broadcast onto that sample's partitions
                    nc.scalar.activation(
                        out=st["Wsq"][sl], in_=st["W"][sl], func=ACTF.Square
                    )
                    bank = psTB if b == 0 else psBB
                    lT = indB[:, b * NROWS : (b + 1) * NROWS]
                    nc.tensor.matmul(
                        out=bank[0:NROWS, 0:256], lhsT=lT,
                        rhs=st["W"][0:128, 0:256], start=True, stop=True,
                    )
                    nc.tensor.matmul(
                        out=bank[0:NROWS, 256:512], lhsT=lT,
                        rhs=st["Wsq"][0:128, 0:256], start=True, stop=True,
                    )

                def p3():
                    # scalar std/threshold chain on this sample's partitions
                    # (psT rows carry the sample sums broadcast per partition)
                    bank = psTB if b == 0 else psBB
                    sx = pstat.tile([128, 2], F32, name=f"sx{b}")
                    scr = pstat.tile([NROWS, 256], F32, name=f"scr{b}")
                    nc.scalar.activation(
                        out=scr[sl], in_=bank[sl, 0:256], func=ACTF.Copy,
                        accum_out=sx[sl, 0:1],
                    )
                    nc.scalar.activation(
                        out=scr[sl], in_=bank[sl, 256:512], func=ACTF.Copy,
                        accum_out=sx[sl, 1:2],
                    )
                    nc.vector.tensor_scalar_mul(
                        out=sx[sl, 0:1], in0=sx[sl, 0:1], scalar1=float(SMSC)
                    )
                    nc.vector.tensor_scalar_mul(
                        out=sx[sl, 1:2], in0=sx[sl, 1:2], scalar1=float(SMSC * SMSC)
                    )
                    sc = pstat.tile([128, 4], F32, name=f"sc{b}")
                    s2 = sc[sl, 0:1]
                    nc.vector.tensor_tensor(
                        out=s2, in0=sx[sl, 0:1], in1=sx[sl, 0:1], op=ALU.mult
                    )
                    nc.vector.tensor_scalar_mul(out=s2, in0=s2, scalar1=1.0 / float(T))
                    var = sc[sl, 1:2]
                    nc.vector.tensor_tensor(
                        out=var, in0=sx[sl, 1:2], in1=s2, op=ALU.subtract
                    )
                    nc.vector.tensor_scalar_mul(
                        out=var, in0=var, scalar1=1.0 / float(T - 1)
                    )
                    std = sc[sl, 2:3]
                    nc.scalar.activation(out=std, in_=var, func=ACTF.Sqrt)
                    # w2t = w2*(1-std); thrH = 0.7-w2t, thrL = 0.4-w2t
                    w2t = sc[sl, 3:4]
                    nc.vector.tensor_scalar(
                        out=w2t, in0=std, scalar1=-1.0, scalar2=1.0,
                        op0=ALU.mult, op1=ALU.add,
                    )
                    nc.vector.tensor_scalar_mul(out=w2t, in0=w2t, scalar1=float(w2))
                    st["thrS"] = pstat.tile([128, 2], F32, name=f"th{b}")
                    nc.vector.tensor_scalar(
                        out=st["thrS"][sl, 0:1], in0=w2t, scalar1=-1.0, scalar2=0.7,
                        op0=ALU.mult, op1=ALU.add,
                    )
                    nc.vector.tensor_scalar(
                        out=st["thrS"][sl, 1:2], in0=w2t, scalar1=-1.0, scalar2=0.4,
                        op0=ALU.mult, op1=ALU.add,
                    )
                    # local-mean chain
                    st["w5"] = pstat.tile([NROWS, 256], BF16, name=f"w5{b}")
                    ca = pstat.tile([NROWS, 258], BF16, name=f"ca{b}")
                    nc.vector.tensor_tensor(
                        out=ca[sl], in0=st["consH"][sl][:, 0:258],
                        in1=st["consH"][sl][:, 1:259], op=ALU.add,
                    )
                    nc.vector.tensor_tensor(
                        out=ca[sl][:, 0:256], in0=ca[sl][:, 0:256],
                        in1=ca[sl][:, 2:258], op=ALU.add,
                    )
                    nc.vector.tensor_tensor(
                        out=st["w5"][sl], in0=ca[sl][:, 0:256],
                        in1=st["consH"][sl][:, 4:260], op=ALU.add,
                    )
                    st["local"] = pstat.tile([NROWS, 256], BF16, name=f"lc{b}")
                    nc.vector.tensor_tensor(
                        out=st["local"][sl], in0=st["w5"][sl], in1=SMb[sl],
                        op=ALU.mult,
                    )

                def p4():
                    # grads branch
                    st["gr"] = pstat.tile([NROWS, 256], BF16, name=f"gr{b}")
                    nc.vector.tensor_tensor(
                        out=st["gr"][sl], in0=st["consH"][sl][:, 2:258],
                        in1=st["consH"][sl][:, 1:257], op=ALU.subtract,
                    )
                    st["gsq"] = pstat.tile([NROWS, 256], BF16, name=f"gq{b}")
                    nc.vector.tensor_tensor(
                        out=st["gsq"][sl], in0=st["gr"][sl], in1=st["gr"][sl],
                        op=ALU.mult,
                    )
                    st["A"] = pstat.tile([NROWS, 256], BF16, name=f"A{b}")
                    nc.vector.tensor_scalar(
                        out=st["A"][sl], in0=st["gsq"][sl], scalar1=th2,
                        scalar2=None, op0=ALU.is_gt,
                    )
                    # compares
                    st["u"] = pstat.tile([NROWS, 256], BF16, name=f"u{b}")
                    nc.vector.tensor_scalar(
                        out=st["u"][sl], in0=st["local"][sl],
                        scalar1=st["thrS"][sl][:, 0:1], scalar2=None, op0=ALU.is_gt,
                    )
                    st["v"] = pstat.tile([NROWS, 256], BF16, name=f"v{b}")
                    nc.vector.tensor_scalar(
                        out=st["v"][sl], in0=st["local"][sl],
                        scalar1=st["thrS"][sl][:, 1:2], scalar2=None, op0=ALU.is_lt,
                    )

                def p4b():
                    if not KDBG:
                        return
                    nc.sync.dma_start(out=dbg["dbg_Sb"][b], in_=st["Sb"][0:NROWS, 0:260])
                    nc.sync.dma_start(out=dbg["dbg_W"][b], in_=st["W"][0:NROWS, 0:256])
                    nc.sync.dma_start(out=dbg["dbg_thr"][b], in_=st["thrS"][0:NROWS, 0:2])
                    nc.sync.dma_start(
                        out=dbg["dbg_local"][b], in_=st["local"][0:NROWS, 0:256]
                    )
                    nc.sync.dma_start(
                        out=dbg["dbg_consH"][b], in_=st["consH"][0:NROWS, 0:260]
                    )
                    nc.sync.dma_start(out=dbg["dbg_A"][b], in_=st["A"][0:NROWS, 0:256])
                    nc.sync.dma_start(out=dbg["dbg_Qs"][b], in_=st["Qs"][0:NROWS, 0:261])
                    nc.sync.dma_start(out=dbg["dbg_cos"][b], in_=st["cos"][0:NROWS, 0:260])
                    nc.sync.dma_start(out=dbg["dbg_t1sv"][b], in_=t1sv[0:NROWS, 0:260])
                    nc.sync.dma_start(
                        out=dbg["dbg_den2"][b], in_=st["den2"][0:NROWS, 0:260]
                    )
                    nc.sync.dma_start(out=dbg["dbg_sd"][b], in_=st["sd"][0:NROWS, 0:260])
                    nc.sync.dma_start(out=dbg["dbg_rs"][b], in_=st["rs"][0:NROWS, 0:260])
                    Dsd = pstat.tile([NROWS, 260], F32, name=f"Dsd{b}")
                    nc.scalar.activation(
                        out=Dsd[sl], in_=psQD[b][sl, 512:772], func=ACTF.Copy
                    )
                    nc.sync.dma_start(out=dbg["dbg_Ds"][b], in_=Dsd[0:NROWS, 0:260])

                def p5():
                    # up = v&gate, dn = u&gate ; act0 = g05|A ; masks f32
                    st["up"] = pstat.tile([NROWS, 256], BF16, name=f"up{b}")
                    nc.vector.tensor_tensor(
                        out=st["up"][sl], in0=st["v"][sl], in1=gate01[sl],
                        op=ALU.mult,
                    )
                    st["dn"] = pstat.tile([NROWS, 256], BF16, name=f"dnm{b}")
                    nc.vector.tensor_tensor(
                        out=st["dn"][sl], in0=st["u"][sl], in1=gate01[sl],
                        op=ALU.mult,
                    )
                    st["act0"] = pstat.tile([NROWS, 256], BF16, name=f"a0{b}")
                    nc.vector.tensor_tensor(
                        out=st["act0"][sl], in0=g05b[sl], in1=st["A"][sl],
                        op=ALU.max,
                    )
                    st["nA"] = pstat.tile([NROWS, 256], BF16, name=f"nA{b}")
                    nc.vector.tensor_scalar(
                        out=st["nA"][sl], in0=st["A"][sl], scalar1=-1.0,
                        scalar2=1.0, op0=ALU.mult, op1=ALU.add,
                    )
                    st["ng"] = pstat.tile([NROWS, 256], BF16, name=f"ng{b}")
                    nc.vector.tensor_tensor(
                        out=st["ng"][sl], in0=st["nA"][sl], in1=g05b[sl],
                        op=ALU.mult,
                    )

                def p6():
                    # copy_predicated is also run full-span at base 0 (custom
                    # partition-sensitive op); masks are zeroed outside [sl]
                    mU, mDA, mD0 = st["mU"], st["mDA"], st["mD0"]
                    nc.vector.tensor_tensor(
                        out=mU[sl], in0=st["up"][sl], in1=st["act0"][sl],
                        op=ALU.mult,
                    )
                    nc.vector.tensor_tensor(
                        out=mDA[sl], in0=st["dn"][sl], in1=st["A"][sl],
                        op=ALU.mult,
                    )
                    nc.vector.tensor_tensor(
                        out=mD0[sl], in0=st["dn"][sl], in1=st["ng"][sl],
                        op=ALU.mult,
                    )
                    fs = slice(0, NROWS)
                    nc.vector.copy_predicated(
                        out=r[fs], mask=mU[fs].bitcast(mybir.dt.int32), data=rU[fs]
                    )
                    nc.vector.copy_predicated(
                        out=r[fs], mask=mDA[fs].bitcast(mybir.dt.int32), data=rDA[fs]
                    )
                    nc.vector.copy_predicated(
                        out=r[fs], mask=mD0[fs].bitcast(mybir.dt.int32),
                        data=rD0_f[fs],
                    )

                def p7():
                    ob = out[b]
                    eng = nc.scalar if b == 0 else nc.sync
                    eng.dma_start(
                        out=bass.AP(
                            tensor=ob.tensor, offset=ob.offset,
                            ap=[[256, 46], [1, 256]],
                        ),
                        in_=r[r0 : r0 + 46, :],
                    )
                    eng.dma_start(
                        out=bass.AP(
                            tensor=ob.tensor, offset=ob.offset + 256 * 46,
                            ap=[[256, 1], [1, LASTW]],
                        ),
                        in_=r[r0 + 46 : r0 + 47, 0:LASTW],
                    )

                return [p1, p2, p3, p4, p4b, p5, p6, p7]

            # ---------------- emission schedule ----------------
            # stream sample 0, with spec/r precompute interleaved
            g0 = 0
            for j, ng in enumerate(NGS):
                emit_tile(0, j, g0, ng)
                g0 += ng

            emit_consts_late()
            emit_smalls()
            pre0 = prep_tail(0)
            pre1 = prep_tail(1)

            tail0 = make_tail(0, pre0)
            # stream sample 1 with tail-0 pieces interleaved
            g0 = 0
            for j, ng in enumerate(NGS):
                emit_tile(1, j, g0, ng)
                g0 += ng
                if j >= 1 and tail0:
                    tail0.pop(0)()
            while tail0:
                tail0.pop(0)()

            for p in make_tail(1, pre1):
                p()

    nc.compile()
    return nc


_CACHE = {}


def _get_nc(wbytes):
    if wbytes not in _CACHE:
        w = np.frombuffer(wbytes, np.float32)
        _CACHE[wbytes] = build_nc(float(w[0]), float(w[1]), float(w[2]))
    return _CACHE[wbytes]


def kernel(**inputs):
    mel = np.ascontiguousarray(np.asarray(inputs["mel_features"], np.float32))
    spec = np.ascontiguousarray(np.asarray(inputs["spectral_features"], np.float32))
    init = np.ascontiguousarray(np.asarray(inputs["initial_boundaries"], np.float32))
    sw = np.asarray(inputs["similarity_weights"], np.float32)
    w = _softmax_f32(sw)
    nc = _get_nc(w.tobytes())

    import ml_dtypes

    bf = ml_dtypes.bfloat16
    in_maps = []
    for c in range(NCORES):
        s = slice(c * BPC, (c + 1) * BPC)
        rpk, t1, g05, rUh, rDAh, rD0h = _host_pack(spec[s], init[s], w[1])
        in_maps.append(
            {
                "mel_features": np.ascontiguousarray(mel[s]),
                "rpk": rpk,
                "t1svh": t1.astype(bf),
                "g05h": g05.astype(bf),
                "rUh": rUh,
                "rDAh": rDAh,
                "rD0h": rD0h,
            }
        )
    res = run_bass_kernel_spmd(nc, in_maps, core_ids=list(range(NCORES)))
    global _LAST_RESULT
    _LAST_RESULT = res
    outs = [np.asarray(res.results[c]["out"], np.float32) for c in range(NCORES)]
    return np.concatenate(outs, axis=0)


_LAST_RESULT = None


if __name__ == "__main__":
    nc = build_nc(1 / 3, 1 / 3, 1 / 3)
    ninst = sum(len(b.instructions) for b in nc.m.functions[0].blocks)
    print("built ok, instructions:", ninst)


# revision 27
# speedup vs baseline: 1.0789x; 1.0183x over previous
"""Trainium2 Bass kernel for nn_AdaptiveBoundaryRefinement_45861660787095.

Self-contained: takes FULL inputs (B=16,M=128,T=12000), shards batch across 8
NeuronCores (2 samples/core), runs a Bass/Tile kernel per core, gathers.

v3 design (v2 tail numerics kept verbatim; front-end rebuilt from trace data):
- mel streams as RAW F32 over two DMA queues (sync HWDGE + gpsimd SWDGE,
  alternating tiles).  The v2 f32->bf16 cast-DMA saturated all 16 SDMA
  engines at only ~280 GB/s read-side; plain f32 streams at the ~358 GB/s
  HBM ceiling and frees the cast entirely.
- S (column sums) matmuls stream the f32 data directly as f32r: with moving
  dim >=256 the PE streams f32r at 1 cycle/row, so no bf16 copy of mel is
  needed anywhere.
- sq = mel^2 moves to the Scalar engine (ACT Square, f32-in/bf16-out,
  1 elem/lane/cycle) - the DVE was the measured critical resource (50.5us
  busy, 54% occ) and TT tops out at 2x regardless of dtype.
- cross = mel[t]*mel[t+1] stays on DVE (f32 1x) except a few big tiles that
  run on GpSimd, balancing all three elementwise engines under the stream.
- Q and D fuse into ONE matmul per chunk: sq and cross live in one combined
  SBUF buffer (cross at column offset CA) and the rhs is a 2-block strided
  AP [[CA,2],[1,261]]; the out AP [[512,2],[1,261]] writes the Q bank and D
  bank of a 2-bank PSUM tile in one pass.  Halves the MM count -> fewer
  fixed overheads and a denser PE stream (keeps the p-state high).
- Per-sample tail (win5 sums, cos, thresholds, closed-form 5-iteration
  refinement via predicated copies) is byte-identical to the validated v2.
- The batch-global early-stop of the reference is a mathematical no-op.
"""

import os
import sys

import numpy as np

_TRN_REPO = "/opt/trn_rl_repo"
if _TRN_REPO not in sys.path:
    sys.path.insert(0, _TRN_REPO)

import concourse.bass as bass
import concourse.bacc as bacc
import concourse.mybir as mybir
import concourse.tile as tile
from concourse.bass_utils import run_bass_kernel_spmd

F32 = mybir.dt.float32
F32R = mybir.dt.float32r
BF16 = mybir.dt.bfloat16
ALU = mybir.AluOpType
ACTF = mybir.ActivationFunctionType
AX = mybir.AxisListType

B, M, T = 16, 128, 12000
NCORES = 8
BPC = B // NCORES            # samples per core = 2
CH = 256                     # chunk width
NCH = (T + CH - 1) // CH     # 47 chunks per sample
EPS2 = 1e-12
GRAD_THRESH = 0.15
LASTW = T - CH * (NCH - 1)   # 224 real cols in the last chunk
NGS = [1, 4, 8, 8, 8, 8, 8, 2]  # chunks per mel tile (small ramp/tail tiles)
SBASE = (0, 64)              # partition base of each sample's chunk rows
NROWS = SBASE[1] + NCH       # 111
CA = 2056                    # cross region offset inside the combined qd tile
QDW = CA + 2053 + 3          # combined sq|cross tile width (bf16)
# odd-j tiles ride the gpsimd (SWDGE) queue and are cast f32->bf16 during the
# DMA: their cross runs at DVE 2x and their S-matmul streams bf16.  Even-j
# tiles stay f32 on the sync (HWDGE) queue at full HBM rate.


def _softmax_f32(x):
    x = np.asarray(x, np.float32)
    m = np.max(x).astype(np.float32)
    e = np.exp((x - m).astype(np.float32)).astype(np.float32)
    return (e / e.sum(dtype=np.float32).astype(np.float32)).astype(np.float32)


def _const_masks(w0, w1):
    import ml_dtypes

    bf = ml_dtypes.bfloat16
    # w1vC [NROWS, 260]: w1 where col maps to t in [0, T), else 0
    # (col h of chunk p -> t = 256p - 2 + h).  Also t=0 keeps w1 (spec_sim=1
    # there via the specH shift trick).
    w1vC = np.full((NROWS, 260), np.float32(w1), np.float32)
    for r0 in SBASE:
        w1vC[r0, 0:2] = 0.0                       # t < 0
        w1vC[r0 + NCH - 1, LASTW + 2 : 260] = 0.0  # t >= T
    # SMb [NROWS, 256]: 0.2 interior, 0.25 at t=1 and t=T-2 (win4 edges)
    SM = np.full((NROWS, 256), np.float32(0.2), np.float32)
    for r0 in SBASE:
        SM[r0, 1] = 0.25
        SM[r0 + NCH - 1, LASTW - 2] = 0.25
    # gate01 [NROWS, 256]: 1 for t in [1, T-2], else 0
    gate = np.ones((NROWS, 256), np.float32)
    for r0 in SBASE:
        gate[r0, 0] = 0.0
        gate[r0 + NCH - 1, LASTW - 1 : 256] = 0.0
    # wzb one-hot bank [128, 257]: col 128 is ones; slice [128-j : 256-j]
    # puts the hot column at position j (j in 0..127).
    wzb = np.zeros((128, 257), bf)
    wzb[:, 128] = 1.0
    wz32 = np.zeros((128, 257), np.float32)
    wz32[:, 128] = 1.0
    # indB2 [128, NROWS]: block-diagonal ones; out[j,:] = sum of W rows of
    # j's sample, broadcast onto every row j of that sample -> both samples'
    # temporal sums in ONE matmul.
    indB = np.zeros((128, NROWS), bf)
    indB[0:NCH, 0:NCH] = 1.0
    indB[SBASE[1] : SBASE[1] + NCH, SBASE[1] : SBASE[1] + NCH] = 1.0
    return w1vC.astype(bf), SM.astype(bf), gate.astype(bf), wzb, indB, wz32


def _host_pack(spec2, init2, w1):
    """Row-pack the [BPC,T] side inputs into the kernel's [NROWS,*] layouts.

    All arithmetic in f32 to mirror the device ops this replaces."""
    rpk = np.zeros((NROWS, 256), np.float32)
    t1 = np.zeros((NROWS, 260), np.float32)
    g05 = np.zeros((NROWS, 256), np.float32)
    rU = np.zeros((NROWS, 256), np.float32)
    rDA = np.zeros((NROWS, 256), np.float32)
    rD0 = np.zeros((NROWS, 256), np.float32)
    for b in range(BPC):
        r0 = SBASE[b]
        full = np.zeros(NCH * 256, np.float32)
        full[:T] = init2[b]
        R = full.reshape(NCH, 256)
        rpk[r0 : r0 + NCH] = R
        g05[r0 : r0 + NCH] = (R > np.float32(0.5)).astype(np.float32)
        rU[r0 : r0 + NCH] = np.minimum(R + np.float32(0.5), np.float32(1.0))
        rDA[r0 : r0 + NCH] = np.maximum(R - np.float32(0.5), np.float32(0.0))
        y = R * np.float32(10.0) - np.float32(5.0)
        k = np.zeros_like(R)
        for j in range(5):
            k += (y > np.float32(j)).astype(np.float32)
        rD0[r0 : r0 + NCH] = R + np.float32(-0.1) * k
        # t1sv: col h of chunk row rl -> t = 256*rl - 2 + h; w1/(1+|dspec|),
        # spec_sim(0)=1, zero outside [0, T)
        sp = spec2[b]
        ssim = np.ones(T, np.float32)
        ssim[1:] = np.float32(1.0) / (
            np.float32(1.0) + np.abs(sp[1:] - sp[:-1]).astype(np.float32)
        )
        tv = np.zeros(NCH * 256 + 260, np.float32)
        tv[2 : 2 + T] = np.float32(w1) * ssim
        for rl in range(NCH):
            t1[r0 + rl] = tv[256 * rl : 256 * rl + 260]
    return rpk, t1, g05, rU, rDA, rD0


def build_nc(w0, w1, w2):
    nc = bacc.Bacc("TRN2", target_bir_lowering=False, debug=False)
    mel = nc.dram_tensor("mel_features", [BPC, M, T], F32, kind="ExternalInput")
    # host-precomputed, row-packed [NROWS, *] side inputs (see _host_pack)
    rpk_d = nc.dram_tensor("rpk", [NROWS, 256], F32, kind="ExternalInput")
    t1sv_d = nc.dram_tensor("t1svh", [NROWS, 260], BF16, kind="ExternalInput")
    g05_d = nc.dram_tensor("g05h", [NROWS, 256], BF16, kind="ExternalInput")
    rU_d = nc.dram_tensor("rUh", [NROWS, 256], F32, kind="ExternalInput")
    rDA_d = nc.dram_tensor("rDAh", [NROWS, 256], F32, kind="ExternalInput")
    rD0_d = nc.dram_tensor("rD0h", [NROWS, 256], F32, kind="ExternalInput")
    out = nc.dram_tensor("out", [BPC, T], F32, kind="ExternalOutput")
    KDBG = bool(os.environ.get("KDBG"))
    dbg = {}
    if KDBG:
        for nm, w, dt in (
            ("dbg_W", 256, BF16), ("dbg_thr", 2, F32), ("dbg_local", 256, BF16),
            ("dbg_consH", 260, BF16), ("dbg_A", 256, BF16), ("dbg_Sb", 260, BF16),
            ("dbg_Qs", 261, BF16), ("dbg_cos", 260, BF16), ("dbg_t1sv", 260, BF16),
            ("dbg_Ds", 260, F32), ("dbg_den2", 260, BF16), ("dbg_sd", 260, F32),
            ("dbg_rs", 260, F32),
        ):
            dbg[nm] = nc.dram_tensor(nm, [BPC, NROWS, w], dt, kind="ExternalOutput")

    _w1vC_np, SM_np, gate_np, wzb_np, indB_np, wz32_np = _const_masks(w0, w1)
    wzb_d = nc.inline_tensor(wzb_np, name="wzbc")
    wz32_d = nc.inline_tensor(wz32_np, name="wz32c")
    indB_d = nc.inline_tensor(indB_np, name="indB")
    SM_d = nc.inline_tensor(SM_np, name="SMb")
    gate_d = nc.inline_tensor(gate_np, name="gate01")

    th2 = float(np.float32(GRAD_THRESH) * np.float32(GRAD_THRESH))
    SMSC = 0.2 / 128.0

    with tile.TileContext(nc) as tc:
        with (
            tc.tile_pool(name="qd", bufs=6) as pqd,
            tc.tile_pool(name="stat", bufs=1) as pstat,
            tc.tile_pool(name="ps", bufs=1, space="PSUM") as pps,
            tc.tile_pool(name="ps2", bufs=1, space="PSUM") as pps2,
        ):
            # ---------------- persistent mel tiles (all 16 fit in SBUF) ---
            W_LAST = NGS[-1] * CH + 6
            T0_LAST = (NCH - NGS[-1]) * CH
            LASTREAL = T - (T0_LAST - 3)
            melts = []  # [b][j]
            for bb in range(BPC):
                row = []
                for jj, ngj in enumerate(NGS):
                    wj = ngj * CH + 6
                    row.append(pstat.tile([128, wj], F32R, name=f"mel{bb}_{jj}"))
                melts.append(row)

            # constants ride the sync (HWDGE) queue ahead of the first mel
            # tile: tiny bytes, and the idle SP sequencer absorbs the per-DMA
            # issue cost (GpSimd Q7 descgen is ~670ns each and would delay
            # the odd-tile mel stream; ACT must stay DMA-free for sq).
            WZb = pstat.tile([128, 257], BF16)
            nc.sync.dma_start(out=WZb, in_=wzb_d[:, :])
            WZ32 = pstat.tile([128, 257], F32R)
            nc.sync.dma_start(out=WZ32, in_=wz32_d[:, :].bitcast(F32R))

            # all 16 mel tile DMAs issue up-front, in ring order per queue:
            # sync (HWDGE) carries the even f32 tiles, gpsimd (SWDGE) the odd
            # cast-to-bf16 tiles.  No pool recycling -> no issue stalls.
            def _mel_dma(b, j):
                ng = NGS[j]
                g0 = sum(NGS[:j])
                t0 = g0 * CH
                src_lo = t0 - 3
                src_hi = min(T, t0 + ng * CH + 3)
                melt = melts[b][j]
                lo_pad = 3 if j == 0 else 0
                s_lo = src_lo + lo_pad
                w_real = src_hi - s_lo
                eng = nc.sync if j % 2 == 0 else nc.gpsimd
                eng.dma_start(
                    out=melt[:, lo_pad : lo_pad + w_real],
                    in_=mel[b, :, s_lo:src_hi].bitcast(F32R),
                )

            for bb in range(BPC):
                for jj in range(0, len(NGS), 2):
                    _mel_dma(bb, jj)
            for bb in range(BPC):
                for jj in range(1, len(NGS), 2):
                    _mel_dma(bb, jj)

            indB = pstat.tile([128, NROWS], BF16)
            SMb = pstat.tile([NROWS, 256], BF16)
            gate01 = pstat.tile([NROWS, 256], BF16)

            def emit_consts_late():
                nc.sync.dma_start(out=indB, in_=indB_d[:, :])
                nc.sync.dma_start(out=SMb, in_=SM_d[:, :])
                nc.sync.dma_start(out=gate01, in_=gate_d[:, :])

            # dummy Sqrt: forces the sqrt_and_others table
            # (square/abs/copy/identity/sqrt) once, at t~0 - no mid-kernel
            # ACT table reload.  Square (the sq op) lives in the same set.
            dummy = pstat.tile([1, 1], F32)
            nc.vector.memset(dummy, 1.0)
            nc.scalar.activation(out=dummy, in_=dummy, func=ACTF.Sqrt)
            epsT = pstat.tile([128, 1], F32)
            nc.vector.memset(epsT, float(EPS2))

            # zero the persistent first/last tile pads
            for bb in range(BPC):
                nc.vector.memset(melts[bb][0][:, 0:3].bitcast(F32), 0.0)
                nc.vector.memset(
                    melts[bb][-1][:, LASTREAL:W_LAST].bitcast(F32), 0.0
                )

            r = pstat.tile([NROWS, 256], F32)
            t1sv = pstat.tile([NROWS, 260], BF16)
            g05b = pstat.tile([NROWS, 256], BF16)
            rU = pstat.tile([NROWS, 256], F32)
            rDA = pstat.tile([NROWS, 256], F32)
            rD0_f = pstat.tile([NROWS, 256], F32)

            def emit_smalls():
                nc.sync.dma_start(out=r, in_=rpk_d[:, :])
                nc.sync.dma_start(out=t1sv, in_=t1sv_d[:, :])
                nc.sync.dma_start(out=g05b, in_=g05_d[:, :])
                nc.sync.dma_start(out=rU, in_=rU_d[:, :])
                nc.sync.dma_start(out=rDA, in_=rDA_d[:, :])
                nc.sync.dma_start(out=rD0_f, in_=rD0_d[:, :])

            # ---------------- PSUM ----------------
            # Both samples share banks: sample 0 occupies rows 0..46, sample
            # 1 rows 64..110 (disjoint partitions), one accumulation group of
            # 2*NCH matmuls per tensor.
            # psS  [128, 512]: col sums,  S[rg, h], h -> t = 256rg-2+h
            # psQD [128,1024]: bank0 Q[rg, 0:261] (t=256rg-3+h),
            #                  bank1 D[rg, 0:260] at cols 512:772
            psS = pps.tile([128, 512], F32, name="psS")
            psQD = pps.tile([128, 1024], F32, name="psQD")
            psTB = pps2.tile([128, 512], F32)

            def _mm(b, row_local, bank, lhsT_onehot, cols, rhs):
                rg = SBASE[b] + row_local
                lhsT = lhsT_onehot[:, 128 - rg : 256 - rg]
                nc.tensor.matmul(
                    out=bank[0:128, cols], lhsT=lhsT, rhs=rhs,
                    start=(b == 0 and row_local == 0),
                    stop=(b == BPC - 1 and row_local == NCH - 1),
                )

            def emit_s_mm(b, row_local, melt, c0):
                _mm(b, row_local, psS, WZ32, slice(0, 260),
                    melt[:, c0 + 1 : c0 + 261])

            def emit_qd_mm(b, row_local, qd, c0):
                _mm(b, row_local, psQD, WZb, slice(0, 261),
                    qd[:, c0 : c0 + 261])
                _mm(b, row_local, psQD, WZb, slice(512, 772),
                    qd[:, CA + c0 : CA + c0 + 260])

            def emit_tile(b, j, g0, ng):
                wmel = ng * CH + 6
                melt = melts[b][j]
                # S matmuls run straight off the f32 data (f32r stream)
                for k in range(ng):
                    emit_s_mm(b, g0 + k, melt, k * CH)
                # combined sq|cross buffer (bf16): sq at [0,wmel),
                # cross at [CA, CA+wmel-1)
                qd = pqd.tile([128, QDW], BF16, tag="qd")
                wx = wmel - 1
                nc.scalar.activation(
                    out=qd[:, 0:wmel], in_=melt[:, 0:wmel].bitcast(F32),
                    func=ACTF.Square,
                )
                nc.vector.tensor_tensor(
                    out=qd[:, CA : CA + wx],
                    in0=melt[:, 0:wx].bitcast(F32),
                    in1=melt[:, 1 : wx + 1].bitcast(F32), op=ALU.mult,
                )
                for k in range(ng):
                    emit_qd_mm(b, g0 + k, qd, k * CH)

            # ---------------- fused tail (both samples, one chain) --------
            # Samples occupy disjoint partition rows (0..46 / 64..110), so
            # the whole refinement tail runs ONCE over rows [0:NROWS).  Rows
            # 47..63 are dead: psS/psQD are zero there (never written after
            # the start=True reset), masks come out zero, copy_predicated
            # leaves r untouched.
            FS = slice(0, NROWS)

            def prep_tail():
                pre = {}
                pre["W"] = pstat.tile([128, 256], BF16, name="Wt")
                nc.vector.memset(pre["W"], 0.0)
                pre["Wsq"] = pstat.tile([128, 256], BF16, name="Wqt")
                nc.vector.memset(pre["Wsq"], 0.0)
                pre["sd"] = pstat.tile([NROWS, 260], F32, name="sdt")
                nc.vector.memset(pre["sd"], 1.0)
                for nm in ("mU", "mDA", "mD0"):
                    pre[nm] = pstat.tile([NROWS, 256], F32, name=nm)
                    nc.vector.memset(pre[nm], 0.0)
                return pre

            def emit_tail(pre):
                st = dict(pre)
                # --- temporal branch: W = win5(Sb), per-sample sums via PE
                st["Sb"] = pstat.tile([NROWS, 260], BF16, name="Sb")
                nc.scalar.activation(
                    out=st["Sb"][FS], in_=psS[FS, 0:260], func=ACTF.Copy
                )
                wa = pstat.tile([NROWS, 258], BF16, name="wa")
                nc.vector.tensor_tensor(
                    out=wa[FS], in0=st["Sb"][FS][:, 0:258],
                    in1=st["Sb"][FS][:, 1:259], op=ALU.add,
                )
                nc.vector.tensor_tensor(
                    out=wa[FS][:, 0:256], in0=wa[FS][:, 0:256],
                    in1=wa[FS][:, 2:258], op=ALU.add,
                )
                nc.vector.tensor_tensor(
                    out=st["W"][FS], in0=wa[FS][:, 0:256],
                    in1=st["Sb"][FS][:, 4:260], op=ALU.add,
                )
                nc.scalar.activation(
                    out=st["Wsq"][FS], in_=st["W"][FS], func=ACTF.Square
                )
                nc.tensor.matmul(
                    out=psTB[0:NROWS, 0:256], lhsT=indB,
                    rhs=st["W"][0:128, 0:256], start=True, stop=True,
                )
                nc.tensor.matmul(
                    out=psTB[0:NROWS, 256:512], lhsT=indB,
                    rhs=st["Wsq"][0:128, 0:256], start=True, stop=True,
                )
                # std / threshold chain (per-partition scalars)
                sx = pstat.tile([128, 2], F32, name="sx")
                scr = pstat.tile([NROWS, 256], F32, name="scr")
                nc.scalar.activation(
                    out=scr[FS], in_=psTB[FS, 0:256], func=ACTF.Copy,
                    accum_out=sx[FS, 0:1],
                )
                nc.scalar.activation(
                    out=scr[FS], in_=psTB[FS, 256:512], func=ACTF.Copy,
                    accum_out=sx[FS, 1:2],
                )
                nc.vector.tensor_scalar_mul(
                    out=sx[FS, 0:1], in0=sx[FS, 0:1], scalar1=float(SMSC)
                )
                nc.vector.tensor_scalar_mul(
                    out=sx[FS, 1:2], in0=sx[FS, 1:2], scalar1=float(SMSC * SMSC)
                )
                sc = pstat.tile([128, 4], F32, name="sc")
                s2 = sc[FS, 0:1]
                nc.vector.tensor_tensor(
                    out=s2, in0=sx[FS, 0:1], in1=sx[FS, 0:1], op=ALU.mult
                )
                nc.vector.tensor_scalar_mul(out=s2, in0=s2, scalar1=1.0 / float(T))
                var = sc[FS, 1:2]
                nc.vector.tensor_tensor(
                    out=var, in0=sx[FS, 1:2], in1=s2, op=ALU.subtract
                )
                nc.vector.tensor_scalar_mul(
                    out=var, in0=var, scalar1=1.0 / float(T - 1)
                )
                std = sc[FS, 2:3]
                nc.scalar.activation(out=std, in_=var, func=ACTF.Sqrt)
                w2t = sc[FS, 3:4]
                nc.vector.tensor_scalar(
                    out=w2t, in0=std, scalar1=-1.0, scalar2=1.0,
                    op0=ALU.mult, op1=ALU.add,
                )
                nc.vector.tensor_scalar_mul(out=w2t, in0=w2t, scalar1=float(w2))
                thrS = pstat.tile([128, 2], F32, name="thr")
                nc.vector.tensor_scalar(
                    out=thrS[FS, 0:1], in0=w2t, scalar1=-1.0, scalar2=0.7,
                    op0=ALU.mult, op1=ALU.add,
                )
                nc.vector.tensor_scalar(
                    out=thrS[FS, 1:2], in0=w2t, scalar1=-1.0, scalar2=0.4,
                    op0=ALU.mult, op1=ALU.add,
                )
                # --- cos branch
                Qs = pstat.tile([NROWS, 261], BF16, name="Qs")
                nc.scalar.activation(
                    out=Qs[FS], in_=psQD[FS, 0:261], func=ACTF.Copy
                )
                den2 = pstat.tile([NROWS, 260], BF16, name="dn")
                nc.vector.tensor_tensor(
                    out=den2[FS], in0=Qs[FS][:, 0:260], in1=Qs[FS][:, 1:261],
                    op=ALU.mult,
                )
                nc.scalar.activation(
                    out=st["sd"][FS], in_=den2[FS], func=ACTF.Sqrt, bias=epsT[FS]
                )
                rs = pstat.tile([NROWS, 260], F32, name="rs")
                nc.vector.reciprocal_approx_fast(out=rs[FS], in_=st["sd"][FS])
                cosb = pstat.tile([NROWS, 260], BF16, name="cs")
                nc.vector.tensor_tensor(
                    out=cosb[FS], in0=psQD[FS, 512:772], in1=rs[FS], op=ALU.mult
                )
                consH = pstat.tile([NROWS, 260], BF16, name="ch")
                nc.vector.scalar_tensor_tensor(
                    out=consH[FS], in0=cosb[FS], scalar=float(w0),
                    in1=t1sv[FS], op0=ALU.mult, op1=ALU.add,
                )
                # local-mean chain
                ca = pstat.tile([NROWS, 258], BF16, name="ca")
                nc.vector.tensor_tensor(
                    out=ca[FS], in0=consH[FS][:, 0:258],
                    in1=consH[FS][:, 1:259], op=ALU.add,
                )
                nc.vector.tensor_tensor(
                    out=ca[FS][:, 0:256], in0=ca[FS][:, 0:256],
                    in1=ca[FS][:, 2:258], op=ALU.add,
                )
                w5 = pstat.tile([NROWS, 256], BF16, name="w5")
                nc.vector.tensor_tensor(
                    out=w5[FS], in0=ca[FS][:, 0:256],
                    in1=consH[FS][:, 4:260], op=ALU.add,
                )
                local = pstat.tile([NROWS, 256], BF16, name="lc")
                nc.vector.tensor_tensor(
                    out=local[FS], in0=w5[FS], in1=SMb[FS], op=ALU.mult
                )
                # grads branch
                gr = pstat.tile([NROWS, 256], BF16, name="gr")
                nc.vector.tensor_tensor(
                    out=gr[FS], in0=consH[FS][:, 2:258],
                    in1=consH[FS][:, 1:257], op=ALU.subtract,
                )
                gsq = pstat.tile([NROWS, 256], BF16, name="gq")
                nc.vector.tensor_tensor(
                    out=gsq[FS], in0=gr[FS], in1=gr[FS], op=ALU.mult
                )
                A = pstat.tile([NROWS, 256], BF16, name="A")
                nc.vector.tensor_scalar(
                    out=A[FS], in0=gsq[FS], scalar1=th2, scalar2=None,
                    op0=ALU.is_gt,
                )
                u = pstat.tile([NROWS, 256], BF16, name="u")
                nc.vector.tensor_scalar(
                    out=u[FS], in0=local[FS], scalar1=thrS[FS][:, 0:1],
                    scalar2=None, op0=ALU.is_gt,
                )
                v = pstat.tile([NROWS, 256], BF16, name="v")
                nc.vector.tensor_scalar(
                    out=v[FS], in0=local[FS], scalar1=thrS[FS][:, 1:2],
                    scalar2=None, op0=ALU.is_lt,
                )
                # masks
                up = pstat.tile([NROWS, 256], BF16, name="up")
                nc.vector.tensor_tensor(
                    out=up[FS], in0=v[FS], in1=gate01[FS], op=ALU.mult
                )
                dn = pstat.tile([NROWS, 256], BF16, name="dnm")
                nc.vector.tensor_tensor(
                    out=dn[FS], in0=u[FS], in1=gate01[FS], op=ALU.mult
                )
                act0 = pstat.tile([NROWS, 256], BF16, name="a0")
                nc.vector.tensor_tensor(
                    out=act0[FS], in0=g05b[FS], in1=A[FS], op=ALU.max
                )
                nA = pstat.tile([NROWS, 256], BF16, name="nA")
                nc.vector.tensor_scalar(
                    out=nA[FS], in0=A[FS], scalar1=-1.0, scalar2=1.0,
                    op0=ALU.mult, op1=ALU.add,
                )
                ngm = pstat.tile([NROWS, 256], BF16, name="ngm")
                nc.vector.tensor_tensor(
                    out=ngm[FS], in0=nA[FS], in1=g05b[FS], op=ALU.mult
                )
                nc.vector.tensor_tensor(
                    out=st["mU"][FS], in0=up[FS], in1=act0[FS], op=ALU.mult
                )
                nc.vector.tensor_tensor(
                    out=st["mDA"][FS], in0=dn[FS], in1=A[FS], op=ALU.mult
                )
                nc.vector.tensor_tensor(
                    out=st["mD0"][FS], in0=dn[FS], in1=ngm[FS], op=ALU.mult
                )
                nc.vector.copy_predicated(
                    out=r[FS], mask=st["mU"][FS].bitcast(mybir.dt.int32),
                    data=rU[FS],
                )
                nc.vector.copy_predicated(
                    out=r[FS], mask=st["mDA"][FS].bitcast(mybir.dt.int32),
                    data=rDA[FS],
                )
                nc.vector.copy_predicated(
                    out=r[FS], mask=st["mD0"][FS].bitcast(mybir.dt.int32),
                    data=rD0_f[FS],
                )
                for b in range(BPC):
                    r0 = SBASE[b]
                    ob = out[b]
                    eng = nc.scalar if b == 0 else nc.sync
                    eng.dma_start(
                        out=bass.AP(
                            tensor=ob.tensor, offset=ob.offset,
                            ap=[[256, 46], [1, 256]],
                        ),
                        in_=r[r0 : r0 + 46, :],
                    )
                    eng.dma_start(
                        out=bass.AP(
                            tensor=ob.tensor, offset=ob.offset + 256 * 46,
                            ap=[[256, 1], [1, LASTW]],
                        ),
                        in_=r[r0 + 46 : r0 + 47, 0:LASTW],
                    )

            # ---------------- emission schedule ----------------
            g0 = 0
            for j, ng in enumerate(NGS):
                emit_tile(0, j, g0, ng)
                g0 += ng
            g0 = 0
            for j, ng in enumerate(NGS):
                emit_tile(1, j, g0, ng)
                g0 += ng

            emit_consts_late()
            emit_smalls()
            pre = prep_tail()
            emit_tail(pre)

    nc.compile()
    return nc


_CACHE = {}


def _get_nc(wbytes):
    if wbytes not in _CACHE:
        w = np.frombuffer(wbytes, np.float32)
        _CACHE[wbytes] = build_nc(float(w[0]), float(w[1]), float(w[2]))
    return _CACHE[wbytes]


def kernel(**inputs):
    mel = np.ascontiguousarray(np.asarray(inputs["mel_features"], np.float32))
    spec = np.ascontiguousarray(np.asarray(inputs["spectral_features"], np.float32))
    init = np.ascontiguousarray(np.asarray(inputs["initial_boundaries"], np.float32))
    sw = np.asarray(inputs["similarity_weights"], np.float32)
    w = _softmax_f32(sw)
    nc = _get_nc(w.tobytes())

    import ml_dtypes

    bf = ml_dtypes.bfloat16
    in_maps = []
    for c in range(NCORES):
        s = slice(c * BPC, (c + 1) * BPC)
        rpk, t1, g05, rUh, rDAh, rD0h = _host_pack(spec[s], init[s], w[1])
        in_maps.append(
            {
                "mel_features": np.ascontiguousarray(mel[s]),
                "rpk": rpk,
                "t1svh": t1.astype(bf),
                "g05h": g05.astype(bf),
                "rUh": rUh,
                "rDAh": rDAh,
                "rD0h": rD0h,
            }
        )
    res = run_bass_kernel_spmd(nc, in_maps, core_ids=list(range(NCORES)))
    global _LAST_RESULT
    _LAST_RESULT = res
    outs = [np.asarray(res.results[c]["out"], np.float32) for c in range(NCORES)]
    return np.concatenate(outs, axis=0)


_LAST_RESULT = None


if __name__ == "__main__":
    nc = build_nc(1 / 3, 1 / 3, 1 / 3)
    ninst = sum(len(b.instructions) for b in nc.m.functions[0].blocks)
    print("built ok, instructions:", ninst)


# revision 28
# speedup vs baseline: 1.1023x; 1.0217x over previous
"""Trainium2 Bass kernel for nn_AdaptiveBoundaryRefinement_45861660787095.

Self-contained: takes FULL inputs (B=16,M=128,T=12000), shards batch across 8
NeuronCores (2 samples/core), runs a Bass/Tile kernel per core, gathers.

v3 design (v2 tail numerics kept verbatim; front-end rebuilt from trace data):
- mel streams as RAW F32 over two DMA queues (sync HWDGE + gpsimd SWDGE,
  alternating tiles).  The v2 f32->bf16 cast-DMA saturated all 16 SDMA
  engines at only ~280 GB/s read-side; plain f32 streams at the ~358 GB/s
  HBM ceiling and frees the cast entirely.
- S (column sums) matmuls stream the f32 data directly as f32r: with moving
  dim >=256 the PE streams f32r at 1 cycle/row, so no bf16 copy of mel is
  needed anywhere.
- sq = mel^2 moves to the Scalar engine (ACT Square, f32-in/bf16-out,
  1 elem/lane/cycle) - the DVE was the measured critical resource (50.5us
  busy, 54% occ) and TT tops out at 2x regardless of dtype.
- cross = mel[t]*mel[t+1] stays on DVE (f32 1x) except a few big tiles that
  run on GpSimd, balancing all three elementwise engines under the stream.
- Q and D fuse into ONE matmul per chunk: sq and cross live in one combined
  SBUF buffer (cross at column offset CA) and the rhs is a 2-block strided
  AP [[CA,2],[1,261]]; the out AP [[512,2],[1,261]] writes the Q bank and D
  bank of a 2-bank PSUM tile in one pass.  Halves the MM count -> fewer
  fixed overheads and a denser PE stream (keeps the p-state high).
- Per-sample tail (win5 sums, cos, thresholds, closed-form 5-iteration
  refinement via predicated copies) is byte-identical to the validated v2.
- The batch-global early-stop of the reference is a mathematical no-op.
"""

import os
import sys

import numpy as np

_TRN_REPO = "/opt/trn_rl_repo"
if _TRN_REPO not in sys.path:
    sys.path.insert(0, _TRN_REPO)

import concourse.bass as bass
import concourse.bacc as bacc
import concourse.mybir as mybir
import concourse.tile as tile
from concourse.bass_utils import run_bass_kernel_spmd

F32 = mybir.dt.float32
F32R = mybir.dt.float32r
BF16 = mybir.dt.bfloat16
ALU = mybir.AluOpType
ACTF = mybir.ActivationFunctionType
AX = mybir.AxisListType

B, M, T = 16, 128, 12000
NCORES = 8
BPC = B // NCORES            # samples per core = 2
CH = 256                     # chunk width
NCH = (T + CH - 1) // CH     # 47 chunks per sample
EPS2 = 1e-12
GRAD_THRESH = 0.15
LASTW = T - CH * (NCH - 1)   # 224 real cols in the last chunk
NGS = [1, 4, 8, 8, 8, 8, 8, 2]  # chunks per mel tile (small ramp/tail tiles)
SBASE = (0, 64)              # partition base of each sample's chunk rows
NROWS = SBASE[1] + NCH       # 111
CA = 2056                    # cross region offset inside the combined qd tile
QDW = CA + 2053 + 3          # combined sq|cross tile width (bf16)
# odd-j tiles ride the gpsimd (SWDGE) queue and are cast f32->bf16 during the
# DMA: their cross runs at DVE 2x and their S-matmul streams bf16.  Even-j
# tiles stay f32 on the sync (HWDGE) queue at full HBM rate.


def _softmax_f32(x):
    x = np.asarray(x, np.float32)
    m = np.max(x).astype(np.float32)
    e = np.exp((x - m).astype(np.float32)).astype(np.float32)
    return (e / e.sum(dtype=np.float32).astype(np.float32)).astype(np.float32)


def _const_masks(w0, w1):
    import ml_dtypes

    bf = ml_dtypes.bfloat16
    # w1vC [NROWS, 260]: w1 where col maps to t in [0, T), else 0
    # (col h of chunk p -> t = 256p - 2 + h).  Also t=0 keeps w1 (spec_sim=1
    # there via the specH shift trick).
    w1vC = np.full((NROWS, 260), np.float32(w1), np.float32)
    for r0 in SBASE:
        w1vC[r0, 0:2] = 0.0                       # t < 0
        w1vC[r0 + NCH - 1, LASTW + 2 : 260] = 0.0  # t >= T
    # SMb [NROWS, 256]: 0.2 interior, 0.25 at t=1 and t=T-2 (win4 edges)
    SM = np.full((NROWS, 256), np.float32(0.2), np.float32)
    for r0 in SBASE:
        SM[r0, 1] = 0.25
        SM[r0 + NCH - 1, LASTW - 2] = 0.25
    # gate01 [NROWS, 256]: 1 for t in [1, T-2], else 0
    gate = np.ones((NROWS, 256), np.float32)
    for r0 in SBASE:
        gate[r0, 0] = 0.0
        gate[r0 + NCH - 1, LASTW - 1 : 256] = 0.0
    # wzb one-hot bank [128, 257]: col 128 is ones; slice [128-j : 256-j]
    # puts the hot column at position j (j in 0..127).
    wzb = np.zeros((128, 257), bf)
    wzb[:, 128] = 1.0
    wz32 = np.zeros((128, 257), np.float32)
    wz32[:, 128] = 1.0
    # indB2 [128, NROWS]: block-diagonal ones; out[j,:] = sum of W rows of
    # j's sample, broadcast onto every row j of that sample -> both samples'
    # temporal sums in ONE matmul.
    indB = np.zeros((128, NROWS), bf)
    indB[0:NCH, 0:NCH] = 1.0
    indB[SBASE[1] : SBASE[1] + NCH, SBASE[1] : SBASE[1] + NCH] = 1.0
    return w1vC.astype(bf), SM.astype(bf), gate.astype(bf), wzb, indB, wz32


def _host_pack(spec2, init2, w1):
    """Row-pack the [BPC,T] side inputs into the kernel's [NROWS,*] layouts.

    All arithmetic in f32 to mirror the device ops this replaces."""
    rpk = np.zeros((NROWS, 256), np.float32)
    t1 = np.zeros((NROWS, 260), np.float32)
    g05 = np.zeros((NROWS, 256), np.float32)
    rU = np.zeros((NROWS, 256), np.float32)
    rDA = np.zeros((NROWS, 256), np.float32)
    rD0 = np.zeros((NROWS, 256), np.float32)
    for b in range(BPC):
        r0 = SBASE[b]
        full = np.zeros(NCH * 256, np.float32)
        full[:T] = init2[b]
        R = full.reshape(NCH, 256)
        rpk[r0 : r0 + NCH] = R
        g05[r0 : r0 + NCH] = (R > np.float32(0.5)).astype(np.float32)
        rU[r0 : r0 + NCH] = np.minimum(R + np.float32(0.5), np.float32(1.0))
        rDA[r0 : r0 + NCH] = np.maximum(R - np.float32(0.5), np.float32(0.0))
        y = R * np.float32(10.0) - np.float32(5.0)
        k = np.zeros_like(R)
        for j in range(5):
            k += (y > np.float32(j)).astype(np.float32)
        rD0[r0 : r0 + NCH] = R + np.float32(-0.1) * k
        # t1sv: col h of chunk row rl -> t = 256*rl - 2 + h; w1/(1+|dspec|),
        # spec_sim(0)=1, zero outside [0, T)
        sp = spec2[b]
        ssim = np.ones(T, np.float32)
        ssim[1:] = np.float32(1.0) / (
            np.float32(1.0) + np.abs(sp[1:] - sp[:-1]).astype(np.float32)
        )
        tv = np.zeros(NCH * 256 + 260, np.float32)
        tv[2 : 2 + T] = np.float32(w1) * ssim
        for rl in range(NCH):
            t1[r0 + rl] = tv[256 * rl : 256 * rl + 260]
    return rpk, t1, g05, rU, rDA, rD0


def build_nc(w0, w1, w2):
    nc = bacc.Bacc("TRN2", target_bir_lowering=False, debug=False)
    mel = nc.dram_tensor("mel_features", [BPC, M, T], F32, kind="ExternalInput")
    # host-precomputed, row-packed [NROWS, *] side inputs (see _host_pack)
    rpk_d = nc.dram_tensor("rpk", [NROWS, 256], F32, kind="ExternalInput")
    t1sv_d = nc.dram_tensor("t1svh", [NROWS, 260], BF16, kind="ExternalInput")
    g05_d = nc.dram_tensor("g05h", [NROWS, 256], BF16, kind="ExternalInput")
    rU_d = nc.dram_tensor("rUh", [NROWS, 256], F32, kind="ExternalInput")
    rDA_d = nc.dram_tensor("rDAh", [NROWS, 256], F32, kind="ExternalInput")
    rD0_d = nc.dram_tensor("rD0h", [NROWS, 256], F32, kind="ExternalInput")
    out = nc.dram_tensor("out", [BPC, T], F32, kind="ExternalOutput")
    KDBG = bool(os.environ.get("KDBG"))
    dbg = {}
    if KDBG:
        for nm, w, dt in (
            ("dbg_W", 256, BF16), ("dbg_thr", 2, F32), ("dbg_local", 256, BF16),
            ("dbg_consH", 260, BF16), ("dbg_A", 256, BF16), ("dbg_Sb", 260, BF16),
            ("dbg_Qs", 261, BF16), ("dbg_cos", 260, BF16), ("dbg_t1sv", 260, BF16),
            ("dbg_Ds", 260, F32), ("dbg_den2", 260, BF16), ("dbg_sd", 260, F32),
            ("dbg_rs", 260, F32),
        ):
            dbg[nm] = nc.dram_tensor(nm, [BPC, NROWS, w], dt, kind="ExternalOutput")

    _w1vC_np, SM_np, gate_np, wzb_np, indB_np, wz32_np = _const_masks(w0, w1)
    wzb_d = nc.inline_tensor(wzb_np, name="wzbc")
    wz32_d = nc.inline_tensor(wz32_np, name="wz32c")
    indB_d = nc.inline_tensor(indB_np, name="indB")
    SM_d = nc.inline_tensor(SM_np, name="SMb")
    gate_d = nc.inline_tensor(gate_np, name="gate01")

    th2 = float(np.float32(GRAD_THRESH) * np.float32(GRAD_THRESH))
    SMSC = 0.2 / 128.0

    with tile.TileContext(nc) as tc:
        with (
            tc.tile_pool(name="qd", bufs=6) as pqd,
            tc.tile_pool(name="stat", bufs=1) as pstat,
            tc.tile_pool(name="ps", bufs=1, space="PSUM") as pps,
            tc.tile_pool(name="ps2", bufs=1, space="PSUM") as pps2,
        ):
            # ---------------- persistent mel tiles (all 16 fit in SBUF) ---
            W_LAST = NGS[-1] * CH + 6
            T0_LAST = (NCH - NGS[-1]) * CH
            LASTREAL = T - (T0_LAST - 3)
            melts = []  # [b][j]
            for bb in range(BPC):
                row = []
                for jj, ngj in enumerate(NGS):
                    wj = ngj * CH + 6
                    row.append(pstat.tile([128, wj], F32R, name=f"mel{bb}_{jj}"))
                melts.append(row)

            # constants ride the sync (HWDGE) queue ahead of the first mel
            # tile: tiny bytes, and the idle SP sequencer absorbs the per-DMA
            # issue cost (GpSimd Q7 descgen is ~670ns each and would delay
            # the odd-tile mel stream; ACT must stay DMA-free for sq).
            WZb = pstat.tile([128, 257], BF16)
            nc.sync.dma_start(out=WZb, in_=wzb_d[:, :])
            WZ32 = pstat.tile([128, 257], F32R)
            nc.sync.dma_start(out=WZ32, in_=wz32_d[:, :].bitcast(F32R))

            # all 16 mel tile DMAs issue up-front, in ring order per queue:
            # sync (HWDGE) carries the even f32 tiles, gpsimd (SWDGE) the odd
            # cast-to-bf16 tiles.  No pool recycling -> no issue stalls.
            def _mel_dma(b, j):
                ng = NGS[j]
                g0 = sum(NGS[:j])
                t0 = g0 * CH
                src_lo = t0 - 3
                src_hi = min(T, t0 + ng * CH + 3)
                melt = melts[b][j]
                lo_pad = 3 if j == 0 else 0
                s_lo = src_lo + lo_pad
                w_real = src_hi - s_lo
                eng = nc.sync if j % 2 == 0 else nc.gpsimd
                eng.dma_start(
                    out=melt[:, lo_pad : lo_pad + w_real],
                    in_=mel[b, :, s_lo:src_hi].bitcast(F32R),
                )

            for bb in range(BPC):
                for jj in range(0, len(NGS), 2):
                    _mel_dma(bb, jj)
            for bb in range(BPC):
                for jj in range(1, len(NGS), 2):
                    _mel_dma(bb, jj)

            indB = pstat.tile([128, NROWS], BF16)
            SMb = pstat.tile([NROWS, 256], BF16)
            gate01 = pstat.tile([NROWS, 256], BF16)

            def emit_consts_late():
                nc.sync.dma_start(out=indB, in_=indB_d[:, :])
                nc.sync.dma_start(out=SMb, in_=SM_d[:, :])
                nc.sync.dma_start(out=gate01, in_=gate_d[:, :])

            # dummy Sqrt: forces the sqrt_and_others table
            # (square/abs/copy/identity/sqrt) once, at t~0 - no mid-kernel
            # ACT table reload.  Square (the sq op) lives in the same set.
            dummy = pstat.tile([1, 1], F32)
            nc.vector.memset(dummy, 1.0)
            nc.scalar.activation(out=dummy, in_=dummy, func=ACTF.Sqrt)
            epsT = pstat.tile([128, 1], F32)
            nc.vector.memset(epsT, float(EPS2))

            # zero the persistent first/last tile pads
            for bb in range(BPC):
                nc.vector.memset(melts[bb][0][:, 0:3].bitcast(F32), 0.0)
                nc.vector.memset(
                    melts[bb][-1][:, LASTREAL:W_LAST].bitcast(F32), 0.0
                )

            r = pstat.tile([NROWS, 256], F32)
            t1sv = pstat.tile([NROWS, 260], BF16)
            g05b = pstat.tile([NROWS, 256], BF16)
            rU = pstat.tile([NROWS, 256], F32)
            rDA = pstat.tile([NROWS, 256], F32)
            rD0_f = pstat.tile([NROWS, 256], F32)

            def emit_smalls():
                nc.sync.dma_start(out=r, in_=rpk_d[:, :])
                nc.sync.dma_start(out=t1sv, in_=t1sv_d[:, :])
                nc.sync.dma_start(out=g05b, in_=g05_d[:, :])
                nc.sync.dma_start(out=rU, in_=rU_d[:, :])
                nc.sync.dma_start(out=rDA, in_=rDA_d[:, :])
                nc.sync.dma_start(out=rD0_f, in_=rD0_d[:, :])

            # ---------------- PSUM ----------------
            # Both samples share banks: sample 0 occupies rows 0..46, sample
            # 1 rows 64..110 (disjoint partitions), one accumulation group of
            # 2*NCH matmuls per tensor.
            # psS  [128, 512]: col sums,  S[rg, h], h -> t = 256rg-2+h
            # psQD [128,1024]: bank0 Q[rg, 0:261] (t=256rg-3+h),
            #                  bank1 D[rg, 0:260] at cols 512:772
            psS = pps.tile([128, 512], F32, name="psS")
            psQD = pps.tile([128, 1024], F32, name="psQD")
            psTB = pps2.tile([128, 512], F32)

            def _mm(b, row_local, bank, lhsT_onehot, cols, rhs):
                rg = SBASE[b] + row_local
                lhsT = lhsT_onehot[:, 128 - rg : 256 - rg]
                nc.tensor.matmul(
                    out=bank[0:128, cols], lhsT=lhsT, rhs=rhs,
                    start=(b == 0 and row_local == 0),
                    stop=(b == BPC - 1 and row_local == NCH - 1),
                )

            def emit_s_mm(b, row_local, melt, c0):
                _mm(b, row_local, psS, WZ32, slice(0, 260),
                    melt[:, c0 + 1 : c0 + 261])

            def emit_qd_mm(b, row_local, qd, c0):
                _mm(b, row_local, psQD, WZb, slice(0, 261),
                    qd[:, c0 : c0 + 261])
                _mm(b, row_local, psQD, WZb, slice(512, 772),
                    qd[:, CA + c0 : CA + c0 + 260])

            # PE p-state warm-up: the HAM only ramps the PE to full clock
            # after ~3us of gap-free execution, and every data-wait gap
            # resets the ramp.  A block of dummy matmuls into the (still
            # unused) psTB bank keeps the PE busy from t~8 so the real chunk
            # MMs run at full clock while tracking the DMA stream.
            def emit_pe_warmup(n_mms):
                for _ in range(n_mms):
                    nc.tensor.matmul(
                        out=psTB[0:128, 0:257], lhsT=WZb[:, 0:128],
                        rhs=WZb[:, 0:257], start=True, stop=True,
                    )

            def emit_tile(b, j, g0, ng):
                wmel = ng * CH + 6
                melt = melts[b][j]
                # S matmuls run straight off the f32 data (f32r stream)
                for k in range(ng):
                    emit_s_mm(b, g0 + k, melt, k * CH)
                # combined sq|cross buffer (bf16): sq at [0,wmel),
                # cross at [CA, CA+wmel-1)
                qd = pqd.tile([128, QDW], BF16, tag="qd")
                wx = wmel - 1
                nc.scalar.activation(
                    out=qd[:, 0:wmel], in_=melt[:, 0:wmel].bitcast(F32),
                    func=ACTF.Square,
                )
                nc.vector.tensor_tensor(
                    out=qd[:, CA : CA + wx],
                    in0=melt[:, 0:wx].bitcast(F32),
                    in1=melt[:, 1 : wx + 1].bitcast(F32), op=ALU.mult,
                )
                for k in range(ng):
                    emit_qd_mm(b, g0 + k, qd, k * CH)

            # ---------------- fused tail (both samples, one chain) --------
            # Samples occupy disjoint partition rows (0..46 / 64..110), so
            # the whole refinement tail runs ONCE over rows [0:NROWS).  Rows
            # 47..63 are dead: psS/psQD are zero there (never written after
            # the start=True reset), masks come out zero, copy_predicated
            # leaves r untouched.
            FS = slice(0, NROWS)

            def prep_tail():
                pre = {}
                pre["W"] = pstat.tile([128, 256], BF16, name="Wt")
                nc.vector.memset(pre["W"], 0.0)
                pre["Wsq"] = pstat.tile([128, 256], BF16, name="Wqt")
                nc.vector.memset(pre["Wsq"], 0.0)
                pre["sd"] = pstat.tile([NROWS, 260], F32, name="sdt")
                nc.vector.memset(pre["sd"], 1.0)
                for nm in ("mU", "mDA", "mD0"):
                    pre[nm] = pstat.tile([NROWS, 256], F32, name=nm)
                    nc.vector.memset(pre[nm], 0.0)
                return pre

            def emit_tail(pre):
                st = dict(pre)
                # --- temporal branch: W = win5(Sb), per-sample sums via PE
                st["Sb"] = pstat.tile([NROWS, 260], BF16, name="Sb")
                nc.scalar.activation(
                    out=st["Sb"][FS], in_=psS[FS, 0:260], func=ACTF.Copy
                )
                wa = pstat.tile([NROWS, 258], BF16, name="wa")
                nc.vector.tensor_tensor(
                    out=wa[FS], in0=st["Sb"][FS][:, 0:258],
                    in1=st["Sb"][FS][:, 1:259], op=ALU.add,
                )
                nc.vector.tensor_tensor(
                    out=wa[FS][:, 0:256], in0=wa[FS][:, 0:256],
                    in1=wa[FS][:, 2:258], op=ALU.add,
                )
                nc.vector.tensor_tensor(
                    out=st["W"][FS], in0=wa[FS][:, 0:256],
                    in1=st["Sb"][FS][:, 4:260], op=ALU.add,
                )
                nc.scalar.activation(
                    out=st["Wsq"][FS], in_=st["W"][FS], func=ACTF.Square
                )
                nc.tensor.matmul(
                    out=psTB[0:NROWS, 0:256], lhsT=indB,
                    rhs=st["W"][0:128, 0:256], start=True, stop=True,
                )
                nc.tensor.matmul(
                    out=psTB[0:NROWS, 256:512], lhsT=indB,
                    rhs=st["Wsq"][0:128, 0:256], start=True, stop=True,
                )
                # std / threshold chain (per-partition scalars)
                sx = pstat.tile([128, 2], F32, name="sx")
                scr = pstat.tile([NROWS, 256], F32, name="scr")
                nc.scalar.activation(
                    out=scr[FS], in_=psTB[FS, 0:256], func=ACTF.Copy,
                    accum_out=sx[FS, 0:1],
                )
                nc.scalar.activation(
                    out=scr[FS], in_=psTB[FS, 256:512], func=ACTF.Copy,
                    accum_out=sx[FS, 1:2],
                )
                nc.vector.tensor_scalar_mul(
                    out=sx[FS, 0:1], in0=sx[FS, 0:1], scalar1=float(SMSC)
                )
                nc.vector.tensor_scalar_mul(
                    out=sx[FS, 1:2], in0=sx[FS, 1:2], scalar1=float(SMSC * SMSC)
                )
                sc = pstat.tile([128, 4], F32, name="sc")
                s2 = sc[FS, 0:1]
                nc.vector.tensor_tensor(
                    out=s2, in0=sx[FS, 0:1], in1=sx[FS, 0:1], op=ALU.mult
                )
                nc.vector.tensor_scalar_mul(out=s2, in0=s2, scalar1=1.0 / float(T))
                var = sc[FS, 1:2]
                nc.vector.tensor_tensor(
                    out=var, in0=sx[FS, 1:2], in1=s2, op=ALU.subtract
                )
                nc.vector.tensor_scalar_mul(
                    out=var, in0=var, scalar1=1.0 / float(T - 1)
                )
                std = sc[FS, 2:3]
                nc.scalar.activation(out=std, in_=var, func=ACTF.Sqrt)
                w2t = sc[FS, 3:4]
                nc.vector.tensor_scalar(
                    out=w2t, in0=std, scalar1=-1.0, scalar2=1.0,
                    op0=ALU.mult, op1=ALU.add,
                )
                nc.vector.tensor_scalar_mul(out=w2t, in0=w2t, scalar1=float(w2))
                thrS = pstat.tile([128, 2], F32, name="thr")
                nc.vector.tensor_scalar(
                    out=thrS[FS, 0:1], in0=w2t, scalar1=-1.0, scalar2=0.7,
                    op0=ALU.mult, op1=ALU.add,
                )
                nc.vector.tensor_scalar(
                    out=thrS[FS, 1:2], in0=w2t, scalar1=-1.0, scalar2=0.4,
                    op0=ALU.mult, op1=ALU.add,
                )
                # --- cos branch
                Qs = pstat.tile([NROWS, 261], BF16, name="Qs")
                nc.scalar.activation(
                    out=Qs[FS], in_=psQD[FS, 0:261], func=ACTF.Copy
                )
                den2 = pstat.tile([NROWS, 260], BF16, name="dn")
                nc.vector.tensor_tensor(
                    out=den2[FS], in0=Qs[FS][:, 0:260], in1=Qs[FS][:, 1:261],
                    op=ALU.mult,
                )
                nc.scalar.activation(
                    out=st["sd"][FS], in_=den2[FS], func=ACTF.Sqrt, bias=epsT[FS]
                )
                rs = pstat.tile([NROWS, 260], F32, name="rs")
                nc.vector.reciprocal_approx_fast(out=rs[FS], in_=st["sd"][FS])
                cosb = pstat.tile([NROWS, 260], BF16, name="cs")
                nc.vector.tensor_tensor(
                    out=cosb[FS], in0=psQD[FS, 512:772], in1=rs[FS], op=ALU.mult
                )
                consH = pstat.tile([NROWS, 260], BF16, name="ch")
                nc.vector.scalar_tensor_tensor(
                    out=consH[FS], in0=cosb[FS], scalar=float(w0),
                    in1=t1sv[FS], op0=ALU.mult, op1=ALU.add,
                )
                # local-mean chain
                ca = pstat.tile([NROWS, 258], BF16, name="ca")
                nc.vector.tensor_tensor(
                    out=ca[FS], in0=consH[FS][:, 0:258],
                    in1=consH[FS][:, 1:259], op=ALU.add,
                )
                nc.vector.tensor_tensor(
                    out=ca[FS][:, 0:256], in0=ca[FS][:, 0:256],
                    in1=ca[FS][:, 2:258], op=ALU.add,
                )
                w5 = pstat.tile([NROWS, 256], BF16, name="w5")
                nc.vector.tensor_tensor(
                    out=w5[FS], in0=ca[FS][:, 0:256],
                    in1=consH[FS][:, 4:260], op=ALU.add,
                )
                local = pstat.tile([NROWS, 256], BF16, name="lc")
                nc.vector.tensor_tensor(
                    out=local[FS], in0=w5[FS], in1=SMb[FS], op=ALU.mult
                )
                # grads branch
                gr = pstat.tile([NROWS, 256], BF16, name="gr")
                nc.vector.tensor_tensor(
                    out=gr[FS], in0=consH[FS][:, 2:258],
                    in1=consH[FS][:, 1:257], op=ALU.subtract,
                )
                gsq = pstat.tile([NROWS, 256], BF16, name="gq")
                nc.vector.tensor_tensor(
                    out=gsq[FS], in0=gr[FS], in1=gr[FS], op=ALU.mult
                )
                A = pstat.tile([NROWS, 256], BF16, name="A")
                nc.vector.tensor_scalar(
                    out=A[FS], in0=gsq[FS], scalar1=th2, scalar2=None,
                    op0=ALU.is_gt,
                )
                u = pstat.tile([NROWS, 256], BF16, name="u")
                nc.vector.tensor_scalar(
                    out=u[FS], in0=local[FS], scalar1=thrS[FS][:, 0:1],
                    scalar2=None, op0=ALU.is_gt,
                )
                v = pstat.tile([NROWS, 256], BF16, name="v")
                nc.vector.tensor_scalar(
                    out=v[FS], in0=local[FS], scalar1=thrS[FS][:, 1:2],
                    scalar2=None, op0=ALU.is_lt,
                )
                # masks
                up = pstat.tile([NROWS, 256], BF16, name="up")
                nc.vector.tensor_tensor(
                    out=up[FS], in0=v[FS], in1=gate01[FS], op=ALU.mult
                )
                dn = pstat.tile([NROWS, 256], BF16, name="dnm")
                nc.vector.tensor_tensor(
                    out=dn[FS], in0=u[FS], in1=gate01[FS], op=ALU.mult
                )
                act0 = pstat.tile([NROWS, 256], BF16, name="a0")
                nc.vector.tensor_tensor(
                    out=act0[FS], in0=g05b[FS], in1=A[FS], op=ALU.max
                )
                nA = pstat.tile([NROWS, 256], BF16, name="nA")
                nc.vector.tensor_scalar(
                    out=nA[FS], in0=A[FS], scalar1=-1.0, scalar2=1.0,
                    op0=ALU.mult, op1=ALU.add,
                )
                ngm = pstat.tile([NROWS, 256], BF16, name="ngm")
                nc.vector.tensor_tensor(
                    out=ngm[FS], in0=nA[FS], in1=g05b[FS], op=ALU.mult
                )
                nc.vector.tensor_tensor(
                    out=st["mU"][FS], in0=up[FS], in1=act0[FS], op=ALU.mult
                )
                nc.vector.tensor_tensor(
                    out=st["mDA"][FS], in0=dn[FS], in1=A[FS], op=ALU.mult
                )
                nc.vector.tensor_tensor(
                    out=st["mD0"][FS], in0=dn[FS], in1=ngm[FS], op=ALU.mult
                )
                nc.vector.copy_predicated(
                    out=r[FS], mask=st["mU"][FS].bitcast(mybir.dt.int32),
                    data=rU[FS],
                )
                nc.vector.copy_predicated(
                    out=r[FS], mask=st["mDA"][FS].bitcast(mybir.dt.int32),
                    data=rDA[FS],
                )
                nc.vector.copy_predicated(
                    out=r[FS], mask=st["mD0"][FS].bitcast(mybir.dt.int32),
                    data=rD0_f[FS],
                )
                for b in range(BPC):
                    r0 = SBASE[b]
                    ob = out[b]
                    eng = nc.scalar if b == 0 else nc.sync
                    eng.dma_start(
                        out=bass.AP(
                            tensor=ob.tensor, offset=ob.offset,
                            ap=[[256, 46], [1, 256]],
                        ),
                        in_=r[r0 : r0 + 46, :],
                    )
                    eng.dma_start(
                        out=bass.AP(
                            tensor=ob.tensor, offset=ob.offset + 256 * 46,
                            ap=[[256, 1], [1, LASTW]],
                        ),
                        in_=r[r0 + 46 : r0 + 47, 0:LASTW],
                    )

            # ---------------- emission schedule ----------------
            emit_pe_warmup(50)
            g0 = 0
            for j, ng in enumerate(NGS):
                emit_tile(0, j, g0, ng)
                g0 += ng
            g0 = 0
            for j, ng in enumerate(NGS):
                emit_tile(1, j, g0, ng)
                g0 += ng

            emit_consts_late()
            emit_smalls()
            pre = prep_tail()
            emit_tail(pre)

    nc.compile()
    return nc


_CACHE = {}


def _get_nc(wbytes):
    if wbytes not in _CACHE:
        w = np.frombuffer(wbytes, np.float32)
        _CACHE[wbytes] = build_nc(float(w[0]), float(w[1]), float(w[2]))
    return _CACHE[wbytes]


def kernel(**inputs):
    mel = np.ascontiguousarray(np.asarray(inputs["mel_features"], np.float32))
    spec = np.ascontiguousarray(np.asarray(inputs["spectral_features"], np.float32))
    init = np.ascontiguousarray(np.asarray(inputs["initial_boundaries"], np.float32))
    sw = np.asarray(inputs["similarity_weights"], np.float32)
    w = _softmax_f32(sw)
    nc = _get_nc(w.tobytes())

    import ml_dtypes

    bf = ml_dtypes.bfloat16
    in_maps = []
    for c in range(NCORES):
        s = slice(c * BPC, (c + 1) * BPC)
        rpk, t1, g05, rUh, rDAh, rD0h = _host_pack(spec[s], init[s], w[1])
        in_maps.append(
            {
                "mel_features": np.ascontiguousarray(mel[s]),
                "rpk": rpk,
                "t1svh": t1.astype(bf),
                "g05h": g05.astype(bf),
                "rUh": rUh,
                "rDAh": rDAh,
                "rD0h": rD0h,
            }
        )
    res = run_bass_kernel_spmd(nc, in_maps, core_ids=list(range(NCORES)))
    global _LAST_RESULT
    _LAST_RESULT = res
    outs = [np.asarray(res.results[c]["out"], np.float32) for c in range(NCORES)]
    return np.concatenate(outs, axis=0)


_LAST_RESULT = None


if __name__ == "__main__":
    nc = build_nc(1 / 3, 1 / 3, 1 / 3)
    ninst = sum(len(b.instructions) for b in nc.m.functions[0].blocks)
    print("built ok, instructions:", ninst)
